# revision 45
# baseline (speedup 1.0000x reference)
"""Trainium2 Bass kernel v3 for nn_BasicTransformerBlock (key-frame cross attention).

Reference computation (B=16 frames, S=1024, C=320, H=8 heads, D=40):
    q = x @ Wq.T ; k = x @ Wk.T ; v = x @ Wv.T
    k, v are taken from frame `kf` only and shared by every frame
    out = softmax(q k^T / sqrt(D)) v     (per frame, per head)
    y = out @ Wo.T + bo

Sharding: data-parallel over frames - 8 cores x 2 frames each; K/V computed
redundantly per core (cheap), outputs concatenate. No collectives.

Design (cost-model driven; ~167 us vs 203.5 us for the v1 baseline):
  - ScalarE exp is the hard floor: 128 exps x [128,1024] ~= 133 us. Schedule
    everything else to hide under it; PE is pre-warmed with dummy matmuls so
    the p-state model hits full clock before the first real projection.
  - Units are (frame, head): 16 units x 8 t-chunks; scores st [t128, s1024]
    in a 2-deep psum ping-pong; exp -> f32r probs tiles (ring of 5).
  - Scores run fp8e4 DoubleRow (0.5 cycles/row, halving the biggest PE
    term): q/k are projected to psum, converted to fp8 in the psum->sbuf
    copy, and repacked into the DoubleRow [32, kk, par, s] layout by a
    DMA round-trip through DRAM scratch (partition remap is free in DMA).
    q/k quantization costs ~1.6e-2 relative error (< 2e-2 tolerance).
    Units 0-1 keep plain f32r scores so the repack latency never touches
    the startup critical path. All activation/projection inputs travel in
    bf16 (q/k noise vanishes under the 3% fp8 step; V adds ~0.4%), cutting
    the startup-critical DMA bytes by ~2x; the PV accumulation itself and
    the O-projection stay f32.
  - PV stays exact f32r with the ones-block denominator trick: lhsT
    v_sb [t128, 64v|64ones per head], accumulators [128,512] x 2 per unit.
    PV emission is deferred by a per-unit ladder (5,4,3,2 then 1 t-slots)
    so unit 0's v-projection conversions never stall the exp stream.
  - q/k/v projections in [128,512]-column chunks through a dedicated 1-bank
    psum slot (plus the psv ring in phase A), emitted into per-tt PE gaps
    ahead of their deadlines; DMA arrival order is tuned so the first exp
    fires at ~13 us.
  - normalize: DVE reciprocal of the ones-rows + tensor_mul into oT, split
    per sh so psum slots free progressively.
  - O-projection per (m, sh): 4 matmuls into a pv-ring psum slot + fused
    bias on the copy-out; frame-0 groups spread into frame-1 units, frame-1
    groups pre-staged in unit 15 / drained through the freed scores ring
    with the tail copies on the (by then idle) ScalarE.
  - y^T [C, S] per frame DMAed out; host un-transposes.
"""

import os
import sys

import numpy as np

try:
    import concourse  # noqa: F401
except ImportError:  # pragma: no cover
    for _p in ("/opt/trn_rl_repo", os.path.dirname(os.path.abspath(__file__))):
        if os.path.isdir(os.path.join(_p, "concourse")):
            sys.path.insert(0, _p)
            break

import concourse.mybir as mybir  # noqa: E402
import concourse.tile as tile  # noqa: E402
from concourse import bacc  # noqa: E402
from concourse import bass_utils  # noqa: E402

F32 = mybir.dt.float32
F32R = mybir.dt.float32r
BF16 = mybir.dt.bfloat16
F8 = mybir.dt.float8e4
DR = mybir.MatmulPerfMode.DoubleRow

S = 1024          # sequence length per frame
C = 320           # channels
H = 8             # heads
D = 40            # head dim
DP = 64           # padded head dim
CP = H * DP       # 512, padded channels
NCORES = 8
FPC = 2           # frames per core
SCALE = float(D) ** -0.5

CI = [(0, 128), (128, 128), (256, 64)]    # c_in chunks of 320
CO = [(0, 128), (128, 128), (256, 64)]    # c_out chunks of 320

# exp(s*SCALE) ~= (1 + y(c1 + y(c2 + y*c3)))^2 with y = s*SCALE/2 folded into
# the coefficients; runs on the (otherwise underused) DVE as a custom op so
# part of the softmax exp stream comes off the Activation-engine bottleneck.
_EXPC = (1.0024652, 0.51482491, 0.16152836)
_ALPHA = SCALE / 2.0
EXP_S0 = _EXPC[0] * _ALPHA
EXP_S1 = _EXPC[1] * _ALPHA * _ALPHA
EXP_IMM2 = _EXPC[2] * _ALPHA * _ALPHA * _ALPHA

# (u, tt) score tiles whose exp runs on DVE instead of Act
DVE_TILES = frozenset((u, tt) for u in range(2, 14) for tt in (2, 5))

_OPS_CACHE: list = []


def _register_exp_op():
    if _OPS_CACHE:
        return _OPS_CACHE[0]
    import concourse.dve_ops as dve_ops
    from concourse.dve_spec import Spec, Src0, C0, C1, C2, One, sq

    for op in dve_ops.OPS:
        if op.name == "EXP_POLY_SQ_ANT":
            _OPS_CACHE.append(op)
            return op

    def _exp_ref(in0, in1, c0, c1, c2):
        x = in0.astype(np.float32)
        p = ((x * c2 + c1) * x + c0) * x + 1.0
        return (p * p).astype(np.float32)

    spec = Spec(
        body=sq(((Src0 * C2 + C1) * Src0 + C0) * Src0 + One),
        reference=_exp_ref,
    )
    dve_ops._SUB_OPCODE_FOR_NAME["EXP_POLY_SQ_ANT"] = (
        dve_ops._CUSTOM_DVE_ROW_BASE + len(dve_ops.OPS))
    op = dve_ops.DveOp("EXP_POLY_SQ_ANT", spec, False,
                       {"v3": "0d91af070d61a8d0"})
    dve_ops.OPS.append(op)
    dve_ops.CUSTOM_DVE_SPECS["EXP_POLY_SQ_ANT"] = spec
    _OPS_CACHE.append(op)
    return op


_NC_CACHE: dict = {}
LAST_RESULTS = None


def _build(loop_n: int = 1):
    exp_op = _register_exp_op()
    nc = bacc.Bacc("TRN2", target_bir_lowering=False, debug=False)

    CPAD = 384
    xt0 = nc.dram_tensor("xt0", [CPAD, S], BF16, kind="ExternalInput")
    xtf = nc.dram_tensor("xtf", [FPC, CPAD, S], BF16, kind="ExternalInput")
    wkq = nc.dram_tensor("wkq", [CPAD, CP], BF16, kind="ExternalInput")
    wqb = nc.dram_tensor("wqb", [CPAD, CP], BF16, kind="ExternalInput")
    wvp = nc.dram_tensor("wvp", [CPAD, CP], BF16, kind="ExternalInput")
    wo = nc.dram_tensor("wo", [CP, C], F32R, kind="ExternalInput")
    bo = nc.dram_tensor("bo", [128, 3], F32, kind="ExternalInput")
    yt = nc.dram_tensor("yt", [FPC, C, S], F32, kind="ExternalOutput")
    # fp8 q/k staging scratch: DMA round-trip repacks [2h x 64d, s] psum-row
    # order into the DoubleRow [32, kk, par, s] layout (slots: 4 k + 8 q)
    qk8d = nc.dram_tensor("qk8d", [12, 128, S], F8, kind="Internal")

    with tile.TileContext(nc) as tc:
        with (
            tc.tile_pool(name="pconst", bufs=1) as pconst,
            tc.tile_pool(name="pqk", bufs=1) as pqk,
            tc.tile_pool(name="pvs", bufs=1) as pvs,
            tc.tile_pool(name="pout", bufs=1) as pout,
            tc.tile_pool(name="pp8", bufs=4) as pp8,
            tc.tile_pool(name="prc", bufs=3) as prc,
            tc.tile_pool(name="py", bufs=6) as py,
            tc.tile_pool(name="psb", bufs=2, space="PSUM") as psb,   # scores ring
            tc.tile_pool(name="psv", bufs=3, space="PSUM") as psv,   # pv/y ring
            tc.tile_pool(name="psj", bufs=1, space="PSUM") as psj,   # proj slot
        ):
          for it in range(loop_n):
            P = f"{it}_"

            # ---------- persistent sbuf tiles ----------
            wkq_sb = pconst.tile([128, 3 * CP], BF16, name=f"{P}wkq", tag="wkq")
            wkq_v = wkq_sb[:].rearrange("p (c w) -> p c w", w=CP)
            wk_sb = [wkq_v[0:cn, ci] for ci, (cs, cn) in enumerate(CI)]
            # q-projection weights in bf16 to pair with the bf16 activations
            # (the compiler requires width-matched matmul inputs)
            wqb_sb = pconst.tile([128, 3 * CP], BF16, name=f"{P}wqb", tag="wqb")
            wqb_v = wqb_sb[:].rearrange("p (c w) -> p c w", w=CP)
            wq_sb = [wqb_v[0:cn, ci] for ci, (cs, cn) in enumerate(CI)]
            wv_all = pconst.tile([128, 3 * CP], BF16, name=f"{P}wv", tag="wv")
            wv_v = wv_all[:].rearrange("p (c w) -> p c w", w=CP)
            wv_sb = [wv_v[0:cn, ci] for ci, (cs, cn) in enumerate(CI)]
            x0_all = pconst.tile([128, 3 * S], BF16, name=f"{P}x0", tag="x0")
            x0_v = x0_all[:].rearrange("p (c w) -> p c w", w=S)
            x0_sb = [x0_v[0:cn, ci] for ci, (cs, cn) in enumerate(CI)]
            # q-side activations in bf16: q/k get fp8-quantized for the
            # DoubleRow scores anyway, so bf16 transport noise (~0.4%) is
            # negligible next to the 3% fp8 step; halves the startup DMAs
            xf_all = [
                pconst.tile([128, 3 * S], BF16, name=f"{P}xf{f}", tag=f"xf{f}")
                for f in range(FPC)
            ]
            xf_v = [xf_all[f][:].rearrange("p (c w) -> p c w", w=S) for f in range(FPC)]
            xf_sb = [
                [xf_v[f][0:cn, ci] for ci, (cs, cn) in enumerate(CI)]
                for f in range(FPC)
            ]
            wo_all = pconst.tile([128, 4 * C], F32R, name=f"{P}wo", tag="wo")
            wo_sb = [wo_all[:, cp * C:(cp + 1) * C] for cp in range(4)]
            bo_all = pconst.tile([128, 3], F32, name=f"{P}bo", tag="bo")
            bo_col = [bo_all[0:cn, m:m + 1] for m, (cs, cn) in enumerate(CO)]

            # fp8 packed q/k for DoubleRow scores: [32, kk2, par2, s1024]
            k8 = [pqk.tile([32, 4 * S], F8, name=f"{P}k8_{m}", tag=f"k8{m}") for m in range(4)]
            q8 = [
                [pqk.tile([32, 4 * S], F8, name=f"{P}q8_{f}_{m}", tag="q8", bufs=4) for m in range(4)]
                for f in range(FPC)
            ]
            k8v = [t[:].rearrange("p (kk par s) -> p kk par s", kk=2, par=2) for t in k8]
            q8v = [
                [t[:].rearrange("p (kk par s) -> p kk par s", kk=2, par=2) for t in q8[f]]
                for f in range(FPC)
            ]
            # f32r q/k for units 0-1 (head-pair 0 of frame 0): keeps the fp8
            # repack DMAs off the startup critical path
            kTp0 = pqk.tile([128, S], F32R, name=f"{P}kTp0", tag="kTp0")
            qTp00 = pqk.tile([128, S], F32R, name=f"{P}qTp00", tag="qTp00")
            # v (f32r, exact): [t128, h8 x (64v | 64 ones)] per t-chunk
            v_sb = [pvs.tile([128, H * 128], F32R, name=f"{P}v{tt}", tag=f"v{tt}")
                    for tt in range(8)]
            # PE warm-up tile memset must run FIRST on Pool so the dummy
            # matmuls can start ramping the p-state model immediately
            # (fp8: f32r memsets are invalid ISA; fp8 matmuls ramp the same)
            zwarm = pconst.tile([128, 512], F8, name=f"{P}zwarm", tag="zw")
            nc.gpsimd.memset(zwarm[:], 0.0)
            ones_sb = pconst.tile([128, DP], F32, name=f"{P}ones", tag="ones")
            nc.gpsimd.memset(ones_sb[:], 1.0)
            # ones blocks of v_sb filled once (DVE is idle in phase A)
            for tt in range(8):
                nc.vector.tensor_copy(
                    v_sb[tt][:].rearrange("p (h m) -> p h m", h=H)[:, :, DP:128],
                    ones_sb[:].unsqueeze(1).broadcast_to([128, H, DP]),
                )

            oT = [pout.tile([128, S], F32R, name=f"{P}oT{f}_{hp}", tag="oT", bufs=8)
                  for f in range(FPC) for hp in range(4)]
            oTf = [oT[0:4], oT[4:8]]

            # ---------- DMA emission (priority order) ----------
            def dma_cols(dst_view, dram_ap, c0, c1, width):
                """columns [c0:c1) of a [CPAD, width] dram tensor into the
                3-chunk sbuf view [128, 3, width]."""
                nc.sync.dma_start(
                    dst_view[:, :, c0:c1],
                    dram_ap.rearrange("(c p) w -> p c w", p=128)[:, :, c0:c1],
                )

            nc.sync.dma_start(bo_all[:], bo.ap())
            dma_cols(wqb_v, wqb.ap(), 0, 128, CP)              # wq head-pair 0
            dma_cols(wkq_v, wkq.ap(), 0, 128, CP)              # wk head-pair 0
            dma_cols(x0_v, xt0.ap(), 0, 128, S)                # x0 t 0:128 (tiny)
            dma_cols(xf_v[0], xtf.ap()[0], 0, 512, S)
            dma_cols(xf_v[0], xtf.ap()[0], 512, 1024, S)
            nc.sync.dma_start(wv_v, wvp.ap().rearrange("(c p) w -> p c w", p=128))
            dma_cols(x0_v, xt0.ap(), 128, 512, S)
            dma_cols(x0_v, xt0.ap(), 512, 1024, S)
            dma_cols(wkq_v, wkq.ap(), 128, CP, CP)             # wk rest
            dma_cols(wqb_v, wqb.ap(), 128, CP, CP)             # wq rest
            # xf1 and wo are deferred into unit 1-3 extras so the small fp8
            # repack hops for k8[1]/q8[0][1] aren't stuck behind their
            # transfers in the FIFO DMA queue

            # ---------- building blocks ----------
            def proj_chunk(dst8, dslot, w_tiles, x_tiles, m, sh, pool=None,
                           f32_dst=None, skip_pack=False):
                """One 512-col chunk of a q/k projection: matmul to psum, DVE
                copy to fp8 stage, then a DRAM round-trip that repacks psum-row
                order (par, kk, p) into the DoubleRow layout [32, kk, par, s].
                dst8: packed [32, kk, par, S] view; dslot: scratch dram slot.
                f32_dst: also (or only, with skip_pack) copy to an f32r tile."""
                ps = (pool or psj).tile(
                    [128, 512], F32, name=f"{P}pj{m}{sh}{dslot}",
                    tag="pj" if pool is None else "pv",
                )
                for ci in range(3):
                    nc.tensor.matmul(
                        ps[:],
                        w_tiles[ci][:, m * 128:(m + 1) * 128],
                        x_tiles[ci][:, sh * 512:(sh + 1) * 512],
                        start=(ci == 0),
                        stop=(ci == 2),
                    )
                cols = slice(sh * 512, (sh + 1) * 512)
                if f32_dst is not None:
                    nc.vector.tensor_copy(f32_dst[:, cols], ps[:])
                if skip_pack:
                    return
                stg = pqk.tile([128, 512], F8, name=f"{P}stg{m}{sh}{dslot}",
                               tag="stg", bufs=2)
                nc.vector.tensor_copy(stg[:], ps[:])
                nc.sync.dma_start(qk8d.ap()[dslot, :, cols], stg[:])
                for par in range(2):
                    nc.sync.dma_start(
                        dst8[:, :, par, cols],
                        qk8d.ap()[dslot, par * 64:(par + 1) * 64].rearrange(
                            "(kk p) s -> p kk s", kk=2)[:, :, cols],
                    )

            def vproj(tt, pool=None):
                """v_sb (f32r, exact) for t-chunk tt."""
                ps = (pool or psj).tile([128, 512], F32, name=f"{P}pv{tt}",
                                        tag="pj" if pool is None else "pv")
                for ci in range(3):
                    nc.tensor.matmul(
                        ps[:],
                        x0_sb[ci][:, tt * 128:(tt + 1) * 128],
                        wv_sb[ci][:],
                        start=(ci == 0),
                        stop=(ci == 2),
                    )
                nc.vector.tensor_copy(
                    v_sb[tt][:].rearrange("p (h m) -> p h m", h=H)[:, :, 0:DP],
                    ps[:].rearrange("p (h c) -> p h c", c=DP),
                )

            def oproj_start(f, m, sh):
                """first half of an O-proj group: psum + cp 0-1 matmuls."""
                cos, con = CO[m]
                ps = psv.tile([con, 512], F32, name=f"{P}py{f}{m}{sh}", tag="pv")
                for cp in range(2):
                    nc.tensor.matmul(
                        ps[:],
                        wo_sb[cp][:, cos:cos + con],
                        oTf[f][cp][:, sh * 512:(sh + 1) * 512],
                        start=(cp == 0),
                        stop=False,
                    )
                return ps

            def oproj_finish(f, m, sh, ps):
                cos, con = CO[m]
                for cp in range(2, 4):
                    nc.tensor.matmul(
                        ps[:],
                        wo_sb[cp][:, cos:cos + con],
                        oTf[f][cp][:, sh * 512:(sh + 1) * 512],
                        start=False,
                        stop=(cp == 3),
                    )
                y_sb = py.tile([con, 512], F32, name=f"{P}y{f}{m}{sh}", tag="y")
                nc.vector.tensor_scalar_add(y_sb[:], ps[:], bo_col[m])
                nc.sync.dma_start(
                    yt.ap()[f, cos:cos + con, sh * 512:(sh + 1) * 512], y_sb[:]
                )

            def oproj_group(f, m, sh):
                oproj_finish(f, m, sh, oproj_start(f, m, sh))

            # ---------- extras schedule: (unit, tt) -> list of closures ----------
            sched: dict = {}

            def at(u, tt, fn):
                sched.setdefault((u, tt), []).append(fn)

            # unit 0: v-projections (psv ring, pipelined 3-deep) and the
            # second half of kT head-pair 0
            at(0, 0, lambda: proj_chunk(k8v[0], 0, wk_sb, x0_sb, 0, 0,
                                        f32_dst=kTp0))
            at(0, 1, lambda: proj_chunk(k8v[0], 0, wk_sb, x0_sb, 0, 1,
                                        f32_dst=kTp0))
            at(0, 1, lambda: vproj(0, psv))
            at(0, 1, lambda: vproj(1, psv))
            at(0, 2, lambda: vproj(2, psv))
            at(0, 2, lambda: vproj(3, psv))
            at(0, 3, lambda: vproj(4, psv))
            at(0, 3, lambda: vproj(5, psv))
            at(0, 4, lambda: vproj(6, psv))
            at(0, 4, lambda: vproj(7, psv))
            at(1, 5, lambda: dma_cols(xf_v[1], xtf.ap()[1], 0, 512, S))
            at(2, 5, lambda: dma_cols(xf_v[1], xtf.ap()[1], 512, 1024, S))
            at(3, 5, lambda: nc.sync.dma_start(
                wo_all[:].rearrange("p (cp c) -> p cp c", c=C),
                wo.ap().rearrange("(cp p) c -> p cp c", p=128),
            ))
            # k/q projections for later units; deadline: unit 2*m (k8[m], q8[0][m]),
            # unit 8+2*m (q8[1][m])
            for m in range(1, 4):
                u0 = 2 * m - 2
                at(u0, 4, lambda m=m: proj_chunk(k8v[m], m, wk_sb, x0_sb, m, 0))
                at(u0, 6, lambda m=m: proj_chunk(k8v[m], m, wk_sb, x0_sb, m, 1))
                at(u0, 7, lambda m=m: proj_chunk(q8v[0][m], 4 + m, wq_sb, xf_sb[0], m, 0))
                at(u0 + 1, 1, lambda m=m: proj_chunk(q8v[0][m], 4 + m, wq_sb, xf_sb[0], m, 1))
            for m in range(4):
                u0 = 5 + 2 * m
                at(u0, 1, lambda m=m: proj_chunk(q8v[1][m], 8 + m, wq_sb, xf_sb[1], m, 0))
                at(u0, 3, lambda m=m: proj_chunk(q8v[1][m], 8 + m, wq_sb, xf_sb[1], m, 1))
            # frame-0 O-projection groups spread into frame-1 units (two halves
            # per group so no single PE insertion exceeds the per-tt slack)
            _ostate: dict = {}
            for g in range(6):
                m, sh = divmod(g, 2)
                at(8 + g, 3, lambda m=m, sh=sh: _ostate.__setitem__(
                    (m, sh), oproj_start(0, m, sh)))
                at(8 + g, 5, lambda m=m, sh=sh: oproj_finish(
                    0, m, sh, _ostate.pop((m, sh))))
            # frame-1 sh0 tail groups: pre-stage cp0-2 in unit 15's last slots
            _ytail: dict = {}

            def ypre(m, pool, tag):
                cos, con = CO[m]
                ps = pool.tile([con, 512], F32, name=f"{P}ypre{m}", tag=tag)
                for cp in range(3):
                    nc.tensor.matmul(
                        ps[:], wo_sb[cp][:, cos:cos + con], oTf[1][cp][:, 0:512],
                        start=(cp == 0), stop=False,
                    )
                _ytail[(m, 0)] = ps

            at(15, 5, lambda: ypre(0, psj, "pj"))
            at(15, 6, lambda: ypre(1, psb, "st"))

            # ---------- phase A: PE warm-up + minimal startup projections ----------
            # ~14 dummy matmuls on zeroed sbuf ramp the PE p-state model to
            # full clock while the input DMAs are still in flight; the real
            # phase-A chunks then cost 213 ns instead of 788 ns.
            pswarm = psj.tile([128, 512], F32, name=f"{P}pswarm", tag="pj")
            for _w in range(7):
                nc.tensor.matmul(pswarm[:], zwarm[:, 0:128], zwarm[:],
                                 start=True, stop=True, skip_group_check=True)
            # critical chain: kT head-pair-0 (t 0:512), q head-pair-0 (full s)
            # via f32r (fp8 pack for k8[0] rides along: only frame 1 needs it)
            proj_chunk(q8v[0][0], 4, wq_sb, xf_sb[0], 0, 0, pool=psv,
                       f32_dst=qTp00, skip_pack=True)
            ps_k0 = psj.tile([128, 128], F32, name=f"{P}pjk0t0", tag="pj")
            for ci in range(3):
                nc.tensor.matmul(ps_k0[:], wk_sb[ci][:, 0:128],
                                 x0_sb[ci][:, 0:128],
                                 start=(ci == 0), stop=(ci == 2))
            nc.vector.tensor_copy(kTp0[:, 0:128], ps_k0[:])
            proj_chunk(q8v[0][0], 4, wq_sb, xf_sb[0], 0, 1,
                       f32_dst=qTp00, skip_pack=True)

            # ---------- main unit loop ----------
            # PV emission for unit u is deferred by defer[u] tt-slots past the
            # pair's last exp: unit 0's v8/r8 conversions trail the first
            # scores, so its PVs (and transitively units 1-3, via the psum-
            # ring rotation) slip; from unit 4 on the schedule is the steady
            # state (PV right after each odd exp, normalize at unit end).
            defer = [5, 4, 3, 2] + [1] * 12
            pv_plan: dict = {}
            for uu in range(16):
                for pp in range(4):
                    ue, te = divmod(8 * uu + 2 * pp + defer[uu], 8)
                    pv_plan.setdefault((ue, te), []).append((uu, pp))
            pvt: dict = {}
            p8t: dict = {}
            meta = [(*divmod(uu, H),) for uu in range(16)]  # (f, h)

            def norm_unit(usrc):
                fs, hs = meta[usrc]
                hps, pars = divmod(hs, 2)
                hls = pars * 64
                # one shared reciprocal tile: the sh1 recip's write-after-read
                # hazard on it forces the sh0 multiply to schedule first, which
                # shortens the critical normalize->O-projection chain
                rc = prc.tile([64, 512], F32, name=f"{P}rc{usrc}", tag="rc")
                for sh in range(2):
                    nc.vector.reciprocal(rc[:], pvt[usrc][sh][64:128, :])
                    nc.vector.tensor_mul(
                        oT[fs * 4 + hps][hls:hls + 64, sh * 512:(sh + 1) * 512],
                        pvt[usrc][sh][0:64, :],
                        rc[:],
                    )

            def emit_pv(usrc, pp):
                fs, hs = meta[usrc]
                if pp == 0:
                    pvt[usrc] = [
                        psv.tile([128, 512], F32, name=f"{P}pvac{usrc}{sh}", tag="pv")
                        for sh in range(2)
                    ]
                for kk in range(2):
                    tt = 2 * pp + kk
                    pt = p8t[(usrc, tt)]
                    for sh in range(2):
                        nc.tensor.matmul(
                            pvt[usrc][sh][:],
                            v_sb[tt][:, hs * 128:(hs + 1) * 128],
                            pt[:, sh * 512:(sh + 1) * 512],
                            start=(tt == 0),
                            stop=(tt == 7),
                        )
                if pp == 3:
                    norm_unit(usrc)

            def emit_scores(u, tt):
                f, h = divmod(u, H)
                hp, par = divmod(h, 2)
                hl = par * 64
                st = psb.tile([128, S], F32, name=f"{P}st{u}{tt}", tag="st")
                for sh in range(2):
                    if u < 2:
                        nc.tensor.matmul(
                            st[:, sh * 512:(sh + 1) * 512],
                            kTp0[hl:hl + 64, tt * 128:(tt + 1) * 128],
                            qTp00[hl:hl + 64, sh * 512:(sh + 1) * 512],
                            start=True,
                            stop=True,
                            tile_position=(hl, 0),
                        )
                    else:
                        nc.tensor.matmul(
                            st[:, sh * 512:(sh + 1) * 512],
                            k8v[hp][:, :, par, tt * 128:(tt + 1) * 128],
                            q8v[f][hp][:, :, par, sh * 512:(sh + 1) * 512],
                            start=True,
                            stop=True,
                            perf_mode=DR,
                            skip_group_check=True,
                        )
                return st

            # scores for slot i+1 are emitted right after exp(i) is issued
            # (before the PV/extras PE work), so the exp stream never waits
            # on late matmuls; the 2-deep psb ring supports exactly this.
            slots = [(u, tt) for u in range(16) for tt in range(8)]
            sc_next = emit_scores(*slots[0])
            for i, (u, tt) in enumerate(slots):
                st = sc_next
                pt = pp8.tile([128, S], F32R, name=f"{P}pt_{u}_{tt}",
                              tag="p8", bufs=6)
                p8t[(u, tt)] = pt
                if (u, tt) in DVE_TILES:
                    nc.vector._custom_dve(
                        exp_op, out=pt[:], in0=st[:],
                        s0=EXP_S0, s1=EXP_S1, imm2=EXP_IMM2,
                    )
                else:
                    nc.scalar.activation(
                        pt[:], st[:], mybir.ActivationFunctionType.Exp,
                        scale=SCALE,
                    )
                if (u, tt) in DVE_TILES and i + 1 < len(slots):
                    # run-ahead only across DVE-exp slots: the Act stream's
                    # next scores are ready the moment its previous exp ends
                    sc_next = emit_scores(*slots[i + 1])
                    run_ahead = True
                else:
                    run_ahead = False
                for fn in sched.pop((u, tt), ()):
                    fn()
                for usrc, ppe in pv_plan.pop((u, tt), ()):
                    emit_pv(usrc, ppe)
                if not run_ahead and i + 1 < len(slots):
                    sc_next = emit_scores(*slots[i + 1])

            # ---------- frame-1 O-projection tail ----------
            # sh0 groups were pre-staged (cp0-2) during unit 15; add the cp3
            # matmul (gated on the last normalize) and drain. sh1 groups run
            # full-size through the freed scores ring.
            # sh1 groups: cp0-2 pre-staged immediately through the freeing psv
            # ring (hp0-2 oT halves have been ready since unit 13)
            ytail1 = []
            for m in range(3):
                cos, con = CO[m]
                ps = psv.tile([con, 512], F32, name=f"{P}pyt1{m}1", tag="pv")
                for cp in range(3):
                    nc.tensor.matmul(
                        ps[:], wo_sb[cp][:, cos:cos + con],
                        oTf[1][cp][:, 512:1024],
                        start=(cp == 0), stop=False,
                    )
                ytail1.append(ps)
            # sh0 drains
            for m in range(2):
                cos, con = CO[m]
                ps = _ytail.pop((m, 0))
                nc.tensor.matmul(
                    ps[:], wo_sb[3][:, cos:cos + con], oTf[1][3][:, 0:512],
                    start=False, stop=True,
                )
                y_sb = py.tile([con, 512], F32, name=f"{P}yt1{m}0", tag="y")
                if m == 0:
                    nc.scalar.activation(y_sb[:], ps[:],
                                         mybir.ActivationFunctionType.Identity,
                                         bias=bo_col[m])
                else:
                    nc.vector.tensor_scalar_add(y_sb[:], ps[:], bo_col[m])
                nc.sync.dma_start(yt.ap()[1, cos:cos + con, 0:512], y_sb[:])
            cos, con = CO[2]
            ps = psb.tile([con, 512], F32, name=f"{P}pyt120", tag="st")
            for cp in range(4):
                nc.tensor.matmul(
                    ps[:], wo_sb[cp][:, cos:cos + con], oTf[1][cp][:, 0:512],
                    start=(cp == 0), stop=(cp == 3),
                )
            y_sb = py.tile([con, 512], F32, name=f"{P}yt120", tag="y")
            nc.scalar.activation(y_sb[:], ps[:],
                                 mybir.ActivationFunctionType.Identity,
                                 bias=bo_col[2])
            nc.sync.dma_start(yt.ap()[1, cos:cos + con, 0:512], y_sb[:])
            # sh1 drains
            for m in range(3):
                cos, con = CO[m]
                ps = ytail1[m]
                nc.tensor.matmul(
                    ps[:], wo_sb[3][:, cos:cos + con], oTf[1][3][:, 512:1024],
                    start=False, stop=True,
                )
                y_sb = py.tile([con, 512], F32, name=f"{P}yt1{m}1", tag="y")
                if m % 2 == 0:
                    nc.scalar.activation(y_sb[:], ps[:],
                                         mybir.ActivationFunctionType.Identity,
                                         bias=bo_col[m])
                else:
                    nc.vector.tensor_scalar_add(y_sb[:], ps[:], bo_col[m])
                nc.sync.dma_start(yt.ap()[1, cos:cos + con, 512:1024], y_sb[:])

            assert not sched, f"unscheduled extras: {list(sched)}"
            assert not _ytail, f"unfinished tail groups: {list(_ytail)}"

    nc.compile()
    return nc


def _get_nc(loop_n: int = 1):
    if loop_n not in _NC_CACHE:
        _NC_CACHE[loop_n] = _build(loop_n)
    return _NC_CACHE[loop_n]


_BF16_NP = mybir.dt.np(mybir.dt.bfloat16)


def _pad_heads_cols(wT: np.ndarray) -> np.ndarray:
    """[C, C] (c_in, c_out) -> [C, CP] with each head's 40 cols at h*64."""
    out = np.zeros((C, CP), np.float32)
    out.reshape(C, H, DP)[:, :, :D] = wT.reshape(C, H, D)
    return out


def _prep_inputs(hidden_states, Wq, Wk, Wv, Wo, bo, video_length, k):
    hidden_states = np.asarray(hidden_states, dtype=np.float32)
    B = hidden_states.shape[0]
    assert hidden_states.shape == (B, S, C), hidden_states.shape
    assert B == NCORES * FPC, B
    kf = int(k)
    vl = int(video_length)
    b = B // vl
    assert b == 1, "kernel specialized for batch 1 (b*video_length == B)"

    xT = np.zeros((B, 384, S), np.float32)
    xT[:, :C, :] = hidden_states.transpose(0, 2, 1)
    wk_p = _pad_heads_cols(np.asarray(Wk, np.float32).T)
    wq_p = _pad_heads_cols(np.asarray(Wq, np.float32).T)
    wv_p = _pad_heads_cols(np.asarray(Wv, np.float32).T)
    wkq_p = np.zeros((384, CP), _BF16_NP)
    wkq_p[:C] = wk_p.astype(_BF16_NP)
    wqb_p = np.zeros((384, CP), _BF16_NP)
    wqb_p[:C] = wq_p.astype(_BF16_NP)
    wvp_p = np.zeros((384, CP), _BF16_NP)
    wvp_p[:C] = wv_p.astype(_BF16_NP)
    wo_p = np.zeros((CP, C), np.float32)
    wo_p.reshape(H, DP, C)[:, :D, :] = np.asarray(Wo, np.float32).T.reshape(H, D, C)
    bo_f = np.zeros(384, np.float32)
    bo_f[:C] = np.asarray(bo, np.float32)
    bo_t = np.ascontiguousarray(bo_f.reshape(3, 128).T)

    xt0 = np.ascontiguousarray(xT[kf].astype(_BF16_NP))
    in_maps = []
    for c in range(NCORES):
        in_maps.append(
            {
                "xt0": xt0,
                "xtf": np.ascontiguousarray(
                    xT[c * FPC:(c + 1) * FPC].astype(_BF16_NP)),
                "wkq": wkq_p,
                "wqb": wqb_p,
                "wvp": wvp_p,
                "wo": wo_p,
                "bo": bo_t,
            }
        )
    return in_maps


def _run(inputs: dict, loop_n: int = 1):
    global LAST_RESULTS
    nc = _get_nc(loop_n)
    in_maps = _prep_inputs(**inputs)
    last_exc = None
    for _attempt in range(3):
        try:
            res = bass_utils.run_bass_kernel_spmd(nc, in_maps, core_ids=list(range(NCORES)))
            break
        except Exception as e:  # transient NRT/axon device hiccups
            last_exc = e
            import time as _time
            _time.sleep(2.0)
    else:
        raise last_exc
    LAST_RESULTS = res
    B = NCORES * FPC
    y = np.empty((B, S, C), np.float32)
    for c in range(NCORES):
        y[c * FPC:(c + 1) * FPC] = res.results[c]["yt"].transpose(0, 2, 1)
    return y


def kernel(hidden_states, Wq, Wk, Wv, Wo, bo, video_length, k):
    return _run(
        dict(
            hidden_states=hidden_states,
            Wq=Wq,
            Wk=Wk,
            Wv=Wv,
            Wo=Wo,
            bo=bo,
            video_length=video_length,
            k=k,
        )
    )



# revision 46
# speedup vs baseline: 1.0097x; 1.0097x over previous
"""Trainium2 Bass kernel v3 for nn_BasicTransformerBlock (key-frame cross attention).

Reference computation (B=16 frames, S=1024, C=320, H=8 heads, D=40):
    q = x @ Wq.T ; k = x @ Wk.T ; v = x @ Wv.T
    k, v are taken from frame `kf` only and shared by every frame
    out = softmax(q k^T / sqrt(D)) v     (per frame, per head)
    y = out @ Wo.T + bo

Sharding: data-parallel over frames - 8 cores x 2 frames each; K/V computed
redundantly per core (cheap), outputs concatenate. No collectives.

Design (cost-model driven; ~167 us vs 203.5 us for the v1 baseline):
  - ScalarE exp is the hard floor: 128 exps x [128,1024] ~= 133 us. Schedule
    everything else to hide under it; PE is pre-warmed with dummy matmuls so
    the p-state model hits full clock before the first real projection.
  - Units are (frame, head): 16 units x 8 t-chunks; scores st [t128, s1024]
    in a 2-deep psum ping-pong; exp -> f32r probs tiles (ring of 5).
  - Scores run fp8e4 DoubleRow (0.5 cycles/row, halving the biggest PE
    term): q/k are projected to psum, converted to fp8 in the psum->sbuf
    copy, and repacked into the DoubleRow [32, kk, par, s] layout by a
    DMA round-trip through DRAM scratch (partition remap is free in DMA).
    q/k quantization costs ~1.6e-2 relative error (< 2e-2 tolerance).
    Units 0-1 keep plain f32r scores so the repack latency never touches
    the startup critical path. All activation/projection inputs travel in
    bf16 (q/k noise vanishes under the 3% fp8 step; V adds ~0.4%), cutting
    the startup-critical DMA bytes by ~2x; the PV accumulation itself and
    the O-projection stay f32.
  - PV stays exact f32r with the ones-block denominator trick: lhsT
    v_sb [t128, 64v|64ones per head], accumulators [128,512] x 2 per unit.
    PV emission is deferred by a per-unit ladder (5,4,3,2 then 1 t-slots)
    so unit 0's v-projection conversions never stall the exp stream.
  - q/k/v projections in [128,512]-column chunks through a dedicated 1-bank
    psum slot (plus the psv ring in phase A), emitted into per-tt PE gaps
    ahead of their deadlines; DMA arrival order is tuned so the first exp
    fires at ~13 us.
  - normalize: DVE reciprocal of the ones-rows + tensor_mul into oT, split
    per sh so psum slots free progressively.
  - O-projection per (m, sh): 4 matmuls into a pv-ring psum slot + fused
    bias on the copy-out; frame-0 groups spread into frame-1 units, frame-1
    groups pre-staged in unit 15 / drained through the freed scores ring
    with the tail copies on the (by then idle) ScalarE.
  - y^T [C, S] per frame DMAed out; host un-transposes.
"""

import os
import sys

import numpy as np

try:
    import concourse  # noqa: F401
except ImportError:  # pragma: no cover
    for _p in ("/opt/trn_rl_repo", os.path.dirname(os.path.abspath(__file__))):
        if os.path.isdir(os.path.join(_p, "concourse")):
            sys.path.insert(0, _p)
            break

import concourse.mybir as mybir  # noqa: E402
import concourse.tile as tile  # noqa: E402
from concourse import bacc  # noqa: E402
from concourse import bass_utils  # noqa: E402

F32 = mybir.dt.float32
F32R = mybir.dt.float32r
BF16 = mybir.dt.bfloat16
F8 = mybir.dt.float8e4
DR = mybir.MatmulPerfMode.DoubleRow

S = 1024          # sequence length per frame
C = 320           # channels
H = 8             # heads
D = 40            # head dim
DP = 64           # padded head dim
CP = H * DP       # 512, padded channels
NCORES = 8
FPC = 2           # frames per core
SCALE = float(D) ** -0.5

CI = [(0, 128), (128, 128), (256, 64)]    # c_in chunks of 320
CO = [(0, 128), (128, 128), (256, 64)]    # c_out chunks of 320

# exp(s*SCALE) ~= (1 + y(c1 + y(c2 + y*c3)))^2 with y = s*SCALE/2 folded into
# the coefficients; runs on the (otherwise underused) DVE as a custom op so
# part of the softmax exp stream comes off the Activation-engine bottleneck.
_EXPC = (1.0024652, 0.51482491, 0.16152836)
_ALPHA = SCALE / 2.0
EXP_S0 = _EXPC[0] * _ALPHA
EXP_S1 = _EXPC[1] * _ALPHA * _ALPHA
EXP_IMM2 = _EXPC[2] * _ALPHA * _ALPHA * _ALPHA

# (u, tt) score tiles whose exp runs on DVE instead of Act
DVE_TILES = frozenset(
    [(u, 5) for u in range(2, 14)] + [(u, 2) for u in range(8, 14)])

_OPS_CACHE: list = []


def _register_exp_op():
    if _OPS_CACHE:
        return _OPS_CACHE[0]
    import concourse.dve_ops as dve_ops
    from concourse.dve_spec import Spec, Src0, C0, C1, C2, One, sq

    for op in dve_ops.OPS:
        if op.name == "EXP_POLY_SQ_ANT":
            _OPS_CACHE.append(op)
            return op

    def _exp_ref(in0, in1, c0, c1, c2):
        x = in0.astype(np.float32)
        p = ((x * c2 + c1) * x + c0) * x + 1.0
        return (p * p).astype(np.float32)

    spec = Spec(
        body=sq(((Src0 * C2 + C1) * Src0 + C0) * Src0 + One),
        reference=_exp_ref,
    )
    dve_ops._SUB_OPCODE_FOR_NAME["EXP_POLY_SQ_ANT"] = (
        dve_ops._CUSTOM_DVE_ROW_BASE + len(dve_ops.OPS))
    op = dve_ops.DveOp("EXP_POLY_SQ_ANT", spec, False,
                       {"v3": "0d91af070d61a8d0"})
    dve_ops.OPS.append(op)
    dve_ops.CUSTOM_DVE_SPECS["EXP_POLY_SQ_ANT"] = spec
    _OPS_CACHE.append(op)
    return op


_NC_CACHE: dict = {}
LAST_RESULTS = None


def _build(loop_n: int = 1):
    exp_op = _register_exp_op()
    nc = bacc.Bacc("TRN2", target_bir_lowering=False, debug=False)

    CPAD = 384
    xt0 = nc.dram_tensor("xt0", [CPAD, S], BF16, kind="ExternalInput")
    xtf = nc.dram_tensor("xtf", [FPC, CPAD, S], BF16, kind="ExternalInput")
    wkq = nc.dram_tensor("wkq", [CPAD, CP], BF16, kind="ExternalInput")
    wqb = nc.dram_tensor("wqb", [CPAD, CP], BF16, kind="ExternalInput")
    wvp = nc.dram_tensor("wvp", [CPAD, CP], BF16, kind="ExternalInput")
    wo = nc.dram_tensor("wo", [CP, C], F32R, kind="ExternalInput")
    bo = nc.dram_tensor("bo", [128, 3], F32, kind="ExternalInput")
    yt = nc.dram_tensor("yt", [FPC, C, S], F32, kind="ExternalOutput")
    # fp8 q/k staging scratch: DMA round-trip repacks [2h x 64d, s] psum-row
    # order into the DoubleRow [32, kk, par, s] layout (slots: 4 k + 8 q)
    qk8d = nc.dram_tensor("qk8d", [12, 128, S], F8, kind="Internal")

    with tile.TileContext(nc) as tc:
        with (
            tc.tile_pool(name="pconst", bufs=1) as pconst,
            tc.tile_pool(name="pqk", bufs=1) as pqk,
            tc.tile_pool(name="pvs", bufs=1) as pvs,
            tc.tile_pool(name="pout", bufs=1) as pout,
            tc.tile_pool(name="pp8", bufs=4) as pp8,
            tc.tile_pool(name="prc", bufs=3) as prc,
            tc.tile_pool(name="py", bufs=6) as py,
            tc.tile_pool(name="psb", bufs=2, space="PSUM") as psb,   # scores ring
            tc.tile_pool(name="psv", bufs=3, space="PSUM") as psv,   # pv/y ring
            tc.tile_pool(name="psj", bufs=1, space="PSUM") as psj,   # proj slot
        ):
          for it in range(loop_n):
            P = f"{it}_"

            # ---------- persistent sbuf tiles ----------
            wkq_sb = pconst.tile([128, 3 * CP], BF16, name=f"{P}wkq", tag="wkq")
            wkq_v = wkq_sb[:].rearrange("p (c w) -> p c w", w=CP)
            wk_sb = [wkq_v[0:cn, ci] for ci, (cs, cn) in enumerate(CI)]
            # q-projection weights in bf16 to pair with the bf16 activations
            # (the compiler requires width-matched matmul inputs)
            wqb_sb = pconst.tile([128, 3 * CP], BF16, name=f"{P}wqb", tag="wqb")
            wqb_v = wqb_sb[:].rearrange("p (c w) -> p c w", w=CP)
            wq_sb = [wqb_v[0:cn, ci] for ci, (cs, cn) in enumerate(CI)]
            wv_all = pconst.tile([128, 3 * CP], BF16, name=f"{P}wv", tag="wv")
            wv_v = wv_all[:].rearrange("p (c w) -> p c w", w=CP)
            wv_sb = [wv_v[0:cn, ci] for ci, (cs, cn) in enumerate(CI)]
            x0_all = pconst.tile([128, 3 * S], BF16, name=f"{P}x0", tag="x0")
            x0_v = x0_all[:].rearrange("p (c w) -> p c w", w=S)
            x0_sb = [x0_v[0:cn, ci] for ci, (cs, cn) in enumerate(CI)]
            # q-side activations in bf16: q/k get fp8-quantized for the
            # DoubleRow scores anyway, so bf16 transport noise (~0.4%) is
            # negligible next to the 3% fp8 step; halves the startup DMAs
            xf_all = [
                pconst.tile([128, 3 * S], BF16, name=f"{P}xf{f}", tag=f"xf{f}")
                for f in range(FPC)
            ]
            xf_v = [xf_all[f][:].rearrange("p (c w) -> p c w", w=S) for f in range(FPC)]
            xf_sb = [
                [xf_v[f][0:cn, ci] for ci, (cs, cn) in enumerate(CI)]
                for f in range(FPC)
            ]
            wo_all = pconst.tile([128, 4 * C], F32R, name=f"{P}wo", tag="wo")
            wo_sb = [wo_all[:, cp * C:(cp + 1) * C] for cp in range(4)]
            bo_all = pconst.tile([128, 3], F32, name=f"{P}bo", tag="bo")
            bo_col = [bo_all[0:cn, m:m + 1] for m, (cs, cn) in enumerate(CO)]

            # fp8 packed q/k for DoubleRow scores: [32, kk2, par2, s1024]
            k8 = [pqk.tile([32, 4 * S], F8, name=f"{P}k8_{m}", tag=f"k8{m}") for m in range(4)]
            q8 = [
                [pqk.tile([32, 4 * S], F8, name=f"{P}q8_{f}_{m}", tag="q8", bufs=4) for m in range(4)]
                for f in range(FPC)
            ]
            k8v = [t[:].rearrange("p (kk par s) -> p kk par s", kk=2, par=2) for t in k8]
            q8v = [
                [t[:].rearrange("p (kk par s) -> p kk par s", kk=2, par=2) for t in q8[f]]
                for f in range(FPC)
            ]
            # f32r q/k for units 0-1 (head-pair 0 of frame 0): keeps the fp8
            # repack DMAs off the startup critical path
            kTp0 = pqk.tile([128, S], F32R, name=f"{P}kTp0", tag="kTp0")
            qTp00 = pqk.tile([128, S], F32R, name=f"{P}qTp00", tag="qTp00")
            # v (f32r, exact): [t128, h8 x (64v | 64 ones)] per t-chunk
            v_sb = [pvs.tile([128, H * 128], F32R, name=f"{P}v{tt}", tag=f"v{tt}")
                    for tt in range(8)]
            # PE warm-up tile memset must run FIRST on Pool so the dummy
            # matmuls can start ramping the p-state model immediately
            # (fp8: f32r memsets are invalid ISA; fp8 matmuls ramp the same)
            zwarm = pconst.tile([128, 512], F8, name=f"{P}zwarm", tag="zw")
            nc.gpsimd.memset(zwarm[:], 0.0)
            ones_sb = pconst.tile([128, DP], F32, name=f"{P}ones", tag="ones")
            nc.gpsimd.memset(ones_sb[:], 1.0)
            # ones blocks of v_sb filled once (DVE is idle in phase A)
            for tt in range(8):
                nc.vector.tensor_copy(
                    v_sb[tt][:].rearrange("p (h m) -> p h m", h=H)[:, :, DP:128],
                    ones_sb[:].unsqueeze(1).broadcast_to([128, H, DP]),
                )

            oT = [pout.tile([128, S], F32R, name=f"{P}oT{f}_{hp}", tag="oT", bufs=8)
                  for f in range(FPC) for hp in range(4)]
            oTf = [oT[0:4], oT[4:8]]

            # ---------- DMA emission (priority order) ----------
            def dma_cols(dst_view, dram_ap, c0, c1, width):
                """columns [c0:c1) of a [CPAD, width] dram tensor into the
                3-chunk sbuf view [128, 3, width]."""
                nc.sync.dma_start(
                    dst_view[:, :, c0:c1],
                    dram_ap.rearrange("(c p) w -> p c w", p=128)[:, :, c0:c1],
                )

            nc.sync.dma_start(bo_all[:], bo.ap())
            dma_cols(wqb_v, wqb.ap(), 0, 128, CP)              # wq head-pair 0
            dma_cols(wkq_v, wkq.ap(), 0, 128, CP)              # wk head-pair 0
            dma_cols(x0_v, xt0.ap(), 0, 128, S)                # x0 t 0:128 (tiny)
            dma_cols(xf_v[0], xtf.ap()[0], 0, 512, S)
            dma_cols(xf_v[0], xtf.ap()[0], 512, 1024, S)
            nc.sync.dma_start(wv_v, wvp.ap().rearrange("(c p) w -> p c w", p=128))
            dma_cols(x0_v, xt0.ap(), 128, 512, S)
            dma_cols(x0_v, xt0.ap(), 512, 1024, S)
            dma_cols(wkq_v, wkq.ap(), 128, CP, CP)             # wk rest
            dma_cols(wqb_v, wqb.ap(), 128, CP, CP)             # wq rest
            # xf1 and wo are deferred into unit 1-3 extras so the small fp8
            # repack hops for k8[1]/q8[0][1] aren't stuck behind their
            # transfers in the FIFO DMA queue

            # ---------- building blocks ----------
            def proj_chunk(dst8, dslot, w_tiles, x_tiles, m, sh, pool=None,
                           f32_dst=None, skip_pack=False):
                """One 512-col chunk of a q/k projection: matmul to psum, DVE
                copy to fp8 stage, then a DRAM round-trip that repacks psum-row
                order (par, kk, p) into the DoubleRow layout [32, kk, par, s].
                dst8: packed [32, kk, par, S] view; dslot: scratch dram slot.
                f32_dst: also (or only, with skip_pack) copy to an f32r tile."""
                ps = (pool or psj).tile(
                    [128, 512], F32, name=f"{P}pj{m}{sh}{dslot}",
                    tag="pj" if pool is None else "pv",
                )
                for ci in range(3):
                    nc.tensor.matmul(
                        ps[:],
                        w_tiles[ci][:, m * 128:(m + 1) * 128],
                        x_tiles[ci][:, sh * 512:(sh + 1) * 512],
                        start=(ci == 0),
                        stop=(ci == 2),
                    )
                cols = slice(sh * 512, (sh + 1) * 512)
                if f32_dst is not None:
                    nc.vector.tensor_copy(f32_dst[:, cols], ps[:])
                if skip_pack:
                    return
                stg = pqk.tile([128, 512], F8, name=f"{P}stg{m}{sh}{dslot}",
                               tag="stg", bufs=2)
                nc.vector.tensor_copy(stg[:], ps[:])
                nc.sync.dma_start(qk8d.ap()[dslot, :, cols], stg[:])
                for par in range(2):
                    nc.sync.dma_start(
                        dst8[:, :, par, cols],
                        qk8d.ap()[dslot, par * 64:(par + 1) * 64].rearrange(
                            "(kk p) s -> p kk s", kk=2)[:, :, cols],
                    )

            def vproj(tt, pool=None):
                """v_sb (f32r, exact) for t-chunk tt."""
                ps = (pool or psj).tile([128, 512], F32, name=f"{P}pv{tt}",
                                        tag="pj" if pool is None else "pv")
                for ci in range(3):
                    nc.tensor.matmul(
                        ps[:],
                        x0_sb[ci][:, tt * 128:(tt + 1) * 128],
                        wv_sb[ci][:],
                        start=(ci == 0),
                        stop=(ci == 2),
                    )
                nc.vector.tensor_copy(
                    v_sb[tt][:].rearrange("p (h m) -> p h m", h=H)[:, :, 0:DP],
                    ps[:].rearrange("p (h c) -> p h c", c=DP),
                )

            def oproj_start(f, m, sh):
                """first half of an O-proj group: psum + cp 0-1 matmuls."""
                cos, con = CO[m]
                ps = psv.tile([con, 512], F32, name=f"{P}py{f}{m}{sh}", tag="pv")
                for cp in range(2):
                    nc.tensor.matmul(
                        ps[:],
                        wo_sb[cp][:, cos:cos + con],
                        oTf[f][cp][:, sh * 512:(sh + 1) * 512],
                        start=(cp == 0),
                        stop=False,
                    )
                return ps

            def oproj_finish(f, m, sh, ps):
                cos, con = CO[m]
                for cp in range(2, 4):
                    nc.tensor.matmul(
                        ps[:],
                        wo_sb[cp][:, cos:cos + con],
                        oTf[f][cp][:, sh * 512:(sh + 1) * 512],
                        start=False,
                        stop=(cp == 3),
                    )
                y_sb = py.tile([con, 512], F32, name=f"{P}y{f}{m}{sh}", tag="y")
                nc.vector.tensor_scalar_add(y_sb[:], ps[:], bo_col[m])
                nc.sync.dma_start(
                    yt.ap()[f, cos:cos + con, sh * 512:(sh + 1) * 512], y_sb[:]
                )

            def oproj_group(f, m, sh):
                oproj_finish(f, m, sh, oproj_start(f, m, sh))

            # ---------- extras schedule: (unit, tt) -> list of closures ----------
            sched: dict = {}

            def at(u, tt, fn):
                sched.setdefault((u, tt), []).append(fn)

            # unit 0: v-projections (psv ring, pipelined 3-deep) and the
            # second half of kT head-pair 0
            at(0, 0, lambda: proj_chunk(k8v[0], 0, wk_sb, x0_sb, 0, 0,
                                        f32_dst=kTp0))
            at(0, 1, lambda: proj_chunk(k8v[0], 0, wk_sb, x0_sb, 0, 1,
                                        f32_dst=kTp0))
            at(0, 1, lambda: vproj(0, psv))
            at(0, 1, lambda: vproj(1, psv))
            at(0, 2, lambda: vproj(2, psv))
            at(0, 2, lambda: vproj(3, psv))
            at(0, 3, lambda: vproj(4, psv))
            at(0, 3, lambda: vproj(5, psv))
            at(0, 4, lambda: vproj(6, psv))
            at(0, 4, lambda: vproj(7, psv))
            at(1, 5, lambda: dma_cols(xf_v[1], xtf.ap()[1], 0, 512, S))
            at(2, 5, lambda: dma_cols(xf_v[1], xtf.ap()[1], 512, 1024, S))
            at(3, 5, lambda: nc.sync.dma_start(
                wo_all[:].rearrange("p (cp c) -> p cp c", c=C),
                wo.ap().rearrange("(cp p) c -> p cp c", p=128),
            ))
            # k/q projections for later units; deadline: unit 2*m (k8[m], q8[0][m]),
            # unit 8+2*m (q8[1][m])
            for m in range(1, 4):
                u0 = 2 * m - 2
                at(u0, 4, lambda m=m: proj_chunk(k8v[m], m, wk_sb, x0_sb, m, 0))
                at(u0, 6, lambda m=m: proj_chunk(k8v[m], m, wk_sb, x0_sb, m, 1))
                at(u0, 7, lambda m=m: proj_chunk(q8v[0][m], 4 + m, wq_sb, xf_sb[0], m, 0))
                at(u0 + 1, 1, lambda m=m: proj_chunk(q8v[0][m], 4 + m, wq_sb, xf_sb[0], m, 1))
            for m in range(4):
                u0 = 5 + 2 * m
                at(u0, 1, lambda m=m: proj_chunk(q8v[1][m], 8 + m, wq_sb, xf_sb[1], m, 0))
                at(u0, 3, lambda m=m: proj_chunk(q8v[1][m], 8 + m, wq_sb, xf_sb[1], m, 1))
            # frame-0 O-projection groups spread into frame-1 units (two halves
            # per group so no single PE insertion exceeds the per-tt slack)
            _ostate: dict = {}
            for g in range(6):
                m, sh = divmod(g, 2)
                at(8 + g, 3, lambda m=m, sh=sh: _ostate.__setitem__(
                    (m, sh), oproj_start(0, m, sh)))
                at(8 + g, 5, lambda m=m, sh=sh: oproj_finish(
                    0, m, sh, _ostate.pop((m, sh))))
            # frame-1 sh0 tail groups: pre-stage cp0-2 in unit 15's last slots
            _ytail: dict = {}

            def ypre(m, pool, tag):
                cos, con = CO[m]
                ps = pool.tile([con, 512], F32, name=f"{P}ypre{m}", tag=tag)
                for cp in range(3):
                    nc.tensor.matmul(
                        ps[:], wo_sb[cp][:, cos:cos + con], oTf[1][cp][:, 0:512],
                        start=(cp == 0), stop=False,
                    )
                _ytail[(m, 0)] = ps

            at(15, 5, lambda: ypre(0, psj, "pj"))
            at(15, 6, lambda: ypre(1, psb, "st"))

            # ---------- phase A: PE warm-up + minimal startup projections ----------
            # ~14 dummy matmuls on zeroed sbuf ramp the PE p-state model to
            # full clock while the input DMAs are still in flight; the real
            # phase-A chunks then cost 213 ns instead of 788 ns.
            pswarm = psj.tile([128, 512], F32, name=f"{P}pswarm", tag="pj")
            for _w in range(7):
                nc.tensor.matmul(pswarm[:], zwarm[:, 0:128], zwarm[:],
                                 start=True, stop=True, skip_group_check=True)
            # critical chain: kT head-pair-0 (t 0:512), q head-pair-0 (full s)
            # via f32r (fp8 pack for k8[0] rides along: only frame 1 needs it)
            proj_chunk(q8v[0][0], 4, wq_sb, xf_sb[0], 0, 0, pool=psv,
                       f32_dst=qTp00, skip_pack=True)
            ps_k0 = psj.tile([128, 128], F32, name=f"{P}pjk0t0", tag="pj")
            for ci in range(3):
                nc.tensor.matmul(ps_k0[:], wk_sb[ci][:, 0:128],
                                 x0_sb[ci][:, 0:128],
                                 start=(ci == 0), stop=(ci == 2))
            nc.vector.tensor_copy(kTp0[:, 0:128], ps_k0[:])
            proj_chunk(q8v[0][0], 4, wq_sb, xf_sb[0], 0, 1,
                       f32_dst=qTp00, skip_pack=True)

            # ---------- main unit loop ----------
            # PV emission for unit u is deferred by defer[u] tt-slots past the
            # pair's last exp: unit 0's v8/r8 conversions trail the first
            # scores, so its PVs (and transitively units 1-3, via the psum-
            # ring rotation) slip; from unit 4 on the schedule is the steady
            # state (PV right after each odd exp, normalize at unit end).
            defer = [5, 4, 3, 2] + [1] * 12
            pv_plan: dict = {}
            for uu in range(16):
                for pp in range(4):
                    ue, te = divmod(8 * uu + 2 * pp + defer[uu], 8)
                    pv_plan.setdefault((ue, te), []).append((uu, pp))
            pvt: dict = {}
            p8t: dict = {}
            meta = [(*divmod(uu, H),) for uu in range(16)]  # (f, h)

            def norm_unit(usrc):
                fs, hs = meta[usrc]
                hps, pars = divmod(hs, 2)
                hls = pars * 64
                # one shared reciprocal tile: the sh1 recip's write-after-read
                # hazard on it forces the sh0 multiply to schedule first, which
                # shortens the critical normalize->O-projection chain
                rc = prc.tile([64, 512], F32, name=f"{P}rc{usrc}", tag="rc")
                for sh in range(2):
                    nc.vector.reciprocal(rc[:], pvt[usrc][sh][64:128, :])
                    nc.vector.tensor_mul(
                        oT[fs * 4 + hps][hls:hls + 64, sh * 512:(sh + 1) * 512],
                        pvt[usrc][sh][0:64, :],
                        rc[:],
                    )

            def emit_pv(usrc, pp):
                fs, hs = meta[usrc]
                if pp == 0:
                    pvt[usrc] = [
                        psv.tile([128, 512], F32, name=f"{P}pvac{usrc}{sh}", tag="pv")
                        for sh in range(2)
                    ]
                for kk in range(2):
                    tt = 2 * pp + kk
                    pt = p8t[(usrc, tt)]
                    for sh in range(2):
                        nc.tensor.matmul(
                            pvt[usrc][sh][:],
                            v_sb[tt][:, hs * 128:(hs + 1) * 128],
                            pt[:, sh * 512:(sh + 1) * 512],
                            start=(tt == 0),
                            stop=(tt == 7),
                        )
                if pp == 3:
                    norm_unit(usrc)

            def emit_scores(u, tt):
                f, h = divmod(u, H)
                hp, par = divmod(h, 2)
                hl = par * 64
                st = psb.tile([128, S], F32, name=f"{P}st{u}{tt}", tag="st")
                for sh in range(2):
                    if u < 2:
                        nc.tensor.matmul(
                            st[:, sh * 512:(sh + 1) * 512],
                            kTp0[hl:hl + 64, tt * 128:(tt + 1) * 128],
                            qTp00[hl:hl + 64, sh * 512:(sh + 1) * 512],
                            start=True,
                            stop=True,
                            tile_position=(hl, 0),
                        )
                    else:
                        nc.tensor.matmul(
                            st[:, sh * 512:(sh + 1) * 512],
                            k8v[hp][:, :, par, tt * 128:(tt + 1) * 128],
                            q8v[f][hp][:, :, par, sh * 512:(sh + 1) * 512],
                            start=True,
                            stop=True,
                            perf_mode=DR,
                            skip_group_check=True,
                        )
                return st

            # scores for slot i+1 are emitted right after exp(i) is issued
            # (before the PV/extras PE work), so the exp stream never waits
            # on late matmuls; the 2-deep psb ring supports exactly this.
            slots = [(u, tt) for u in range(16) for tt in range(8)]
            sc_next = emit_scores(*slots[0])
            for i, (u, tt) in enumerate(slots):
                st = sc_next
                pt = pp8.tile([128, S], F32R, name=f"{P}pt_{u}_{tt}",
                              tag="p8", bufs=6)
                p8t[(u, tt)] = pt
                if (u, tt) in DVE_TILES:
                    nc.vector._custom_dve(
                        exp_op, out=pt[:], in0=st[:],
                        s0=EXP_S0, s1=EXP_S1, imm2=EXP_IMM2,
                    )
                else:
                    nc.scalar.activation(
                        pt[:], st[:], mybir.ActivationFunctionType.Exp,
                        scale=SCALE,
                    )
                if (u, tt) in DVE_TILES and i + 1 < len(slots):
                    # run-ahead only across DVE-exp slots: the Act stream's
                    # next scores are ready the moment its previous exp ends
                    sc_next = emit_scores(*slots[i + 1])
                    run_ahead = True
                else:
                    run_ahead = False
                for fn in sched.pop((u, tt), ()):
                    fn()
                for usrc, ppe in pv_plan.pop((u, tt), ()):
                    emit_pv(usrc, ppe)
                if not run_ahead and i + 1 < len(slots):
                    sc_next = emit_scores(*slots[i + 1])

            # ---------- frame-1 O-projection tail ----------
            # sh0 groups were pre-staged (cp0-2) during unit 15; add the cp3
            # matmul (gated on the last normalize) and drain. sh1 groups run
            # full-size through the freed scores ring.
            # sh1 groups: cp0-2 pre-staged immediately through the freeing psv
            # ring (hp0-2 oT halves have been ready since unit 13)
            ytail1 = []
            for m in range(3):
                cos, con = CO[m]
                ps = psv.tile([con, 512], F32, name=f"{P}pyt1{m}1", tag="pv")
                for cp in range(3):
                    nc.tensor.matmul(
                        ps[:], wo_sb[cp][:, cos:cos + con],
                        oTf[1][cp][:, 512:1024],
                        start=(cp == 0), stop=False,
                    )
                ytail1.append(ps)
            # sh0 drains
            for m in range(2):
                cos, con = CO[m]
                ps = _ytail.pop((m, 0))
                nc.tensor.matmul(
                    ps[:], wo_sb[3][:, cos:cos + con], oTf[1][3][:, 0:512],
                    start=False, stop=True,
                )
                y_sb = py.tile([con, 512], F32, name=f"{P}yt1{m}0", tag="y")
                if m == 0:
                    nc.scalar.activation(y_sb[:], ps[:],
                                         mybir.ActivationFunctionType.Identity,
                                         bias=bo_col[m])
                else:
                    nc.vector.tensor_scalar_add(y_sb[:], ps[:], bo_col[m])
                nc.sync.dma_start(yt.ap()[1, cos:cos + con, 0:512], y_sb[:])
            cos, con = CO[2]
            ps = psb.tile([con, 512], F32, name=f"{P}pyt120", tag="st")
            for cp in range(4):
                nc.tensor.matmul(
                    ps[:], wo_sb[cp][:, cos:cos + con], oTf[1][cp][:, 0:512],
                    start=(cp == 0), stop=(cp == 3),
                )
            y_sb = py.tile([con, 512], F32, name=f"{P}yt120", tag="y")
            nc.scalar.activation(y_sb[:], ps[:],
                                 mybir.ActivationFunctionType.Identity,
                                 bias=bo_col[2])
            nc.sync.dma_start(yt.ap()[1, cos:cos + con, 0:512], y_sb[:])
            # sh1 drains
            for m in range(3):
                cos, con = CO[m]
                ps = ytail1[m]
                nc.tensor.matmul(
                    ps[:], wo_sb[3][:, cos:cos + con], oTf[1][3][:, 512:1024],
                    start=False, stop=True,
                )
                y_sb = py.tile([con, 512], F32, name=f"{P}yt1{m}1", tag="y")
                if m % 2 == 0:
                    nc.scalar.activation(y_sb[:], ps[:],
                                         mybir.ActivationFunctionType.Identity,
                                         bias=bo_col[m])
                else:
                    nc.vector.tensor_scalar_add(y_sb[:], ps[:], bo_col[m])
                nc.sync.dma_start(yt.ap()[1, cos:cos + con, 512:1024], y_sb[:])

            assert not sched, f"unscheduled extras: {list(sched)}"
            assert not _ytail, f"unfinished tail groups: {list(_ytail)}"

    nc.compile()
    return nc


def _get_nc(loop_n: int = 1):
    if loop_n not in _NC_CACHE:
        _NC_CACHE[loop_n] = _build(loop_n)
    return _NC_CACHE[loop_n]


_BF16_NP = mybir.dt.np(mybir.dt.bfloat16)


def _pad_heads_cols(wT: np.ndarray) -> np.ndarray:
    """[C, C] (c_in, c_out) -> [C, CP] with each head's 40 cols at h*64."""
    out = np.zeros((C, CP), np.float32)
    out.reshape(C, H, DP)[:, :, :D] = wT.reshape(C, H, D)
    return out


def _prep_inputs(hidden_states, Wq, Wk, Wv, Wo, bo, video_length, k):
    hidden_states = np.asarray(hidden_states, dtype=np.float32)
    B = hidden_states.shape[0]
    assert hidden_states.shape == (B, S, C), hidden_states.shape
    assert B == NCORES * FPC, B
    kf = int(k)
    vl = int(video_length)
    b = B // vl
    assert b == 1, "kernel specialized for batch 1 (b*video_length == B)"

    xT = np.zeros((B, 384, S), np.float32)
    xT[:, :C, :] = hidden_states.transpose(0, 2, 1)
    wk_p = _pad_heads_cols(np.asarray(Wk, np.float32).T)
    wq_p = _pad_heads_cols(np.asarray(Wq, np.float32).T)
    wv_p = _pad_heads_cols(np.asarray(Wv, np.float32).T)
    wkq_p = np.zeros((384, CP), _BF16_NP)
    wkq_p[:C] = wk_p.astype(_BF16_NP)
    wqb_p = np.zeros((384, CP), _BF16_NP)
    wqb_p[:C] = wq_p.astype(_BF16_NP)
    wvp_p = np.zeros((384, CP), _BF16_NP)
    wvp_p[:C] = wv_p.astype(_BF16_NP)
    wo_p = np.zeros((CP, C), np.float32)
    wo_p.reshape(H, DP, C)[:, :D, :] = np.asarray(Wo, np.float32).T.reshape(H, D, C)
    bo_f = np.zeros(384, np.float32)
    bo_f[:C] = np.asarray(bo, np.float32)
    bo_t = np.ascontiguousarray(bo_f.reshape(3, 128).T)

    xt0 = np.ascontiguousarray(xT[kf].astype(_BF16_NP))
    in_maps = []
    for c in range(NCORES):
        in_maps.append(
            {
                "xt0": xt0,
                "xtf": np.ascontiguousarray(
                    xT[c * FPC:(c + 1) * FPC].astype(_BF16_NP)),
                "wkq": wkq_p,
                "wqb": wqb_p,
                "wvp": wvp_p,
                "wo": wo_p,
                "bo": bo_t,
            }
        )
    return in_maps


def _run(inputs: dict, loop_n: int = 1):
    global LAST_RESULTS
    nc = _get_nc(loop_n)
    in_maps = _prep_inputs(**inputs)
    last_exc = None
    for _attempt in range(3):
        try:
            res = bass_utils.run_bass_kernel_spmd(nc, in_maps, core_ids=list(range(NCORES)))
            break
        except Exception as e:  # transient NRT/axon device hiccups
            last_exc = e
            import time as _time
            _time.sleep(2.0)
    else:
        raise last_exc
    LAST_RESULTS = res
    B = NCORES * FPC
    y = np.empty((B, S, C), np.float32)
    for c in range(NCORES):
        y[c * FPC:(c + 1) * FPC] = res.results[c]["yt"].transpose(0, 2, 1)
    return y


def kernel(hidden_states, Wq, Wk, Wv, Wo, bo, video_length, k):
    return _run(
        dict(
            hidden_states=hidden_states,
            Wq=Wq,
            Wk=Wk,
            Wv=Wv,
            Wo=Wo,
            bo=bo,
            video_length=video_length,
            k=k,
        )
    )



# revision 47
# speedup vs baseline: 1.0315x; 1.0216x over previous
"""Trainium2 Bass kernel v3 for nn_BasicTransformerBlock (key-frame cross attention).

Reference computation (B=16 frames, S=1024, C=320, H=8 heads, D=40):
    q = x @ Wq.T ; k = x @ Wk.T ; v = x @ Wv.T
    k, v are taken from frame `kf` only and shared by every frame
    out = softmax(q k^T / sqrt(D)) v     (per frame, per head)
    y = out @ Wo.T + bo

Sharding: data-parallel over frames - 8 cores x 2 frames each; K/V computed
redundantly per core (cheap), outputs concatenate. No collectives.

Design (cost-model driven; ~167 us vs 203.5 us for the v1 baseline):
  - ScalarE exp is the hard floor: 128 exps x [128,1024] ~= 133 us. Schedule
    everything else to hide under it; PE is pre-warmed with dummy matmuls so
    the p-state model hits full clock before the first real projection.
  - Units are (frame, head): 16 units x 8 t-chunks; scores st [t128, s1024]
    in a 2-deep psum ping-pong; exp -> f32r probs tiles (ring of 5).
  - Scores run fp8e4 DoubleRow (0.5 cycles/row, halving the biggest PE
    term): q/k are projected to psum, converted to fp8 in the psum->sbuf
    copy, and repacked into the DoubleRow [32, kk, par, s] layout by a
    DMA round-trip through DRAM scratch (partition remap is free in DMA).
    q/k quantization costs ~1.6e-2 relative error (< 2e-2 tolerance).
    Units 0-1 keep plain f32r scores so the repack latency never touches
    the startup critical path. All activation/projection inputs travel in
    bf16 (q/k noise vanishes under the 3% fp8 step; V adds ~0.4%), cutting
    the startup-critical DMA bytes by ~2x; the PV accumulation itself and
    the O-projection stay f32.
  - PV stays exact f32r with the ones-block denominator trick: lhsT
    v_sb [t128, 64v|64ones per head], accumulators [128,512] x 2 per unit.
    PV emission is deferred by a per-unit ladder (5,4,3,2 then 1 t-slots)
    so unit 0's v-projection conversions never stall the exp stream.
  - q/k/v projections in [128,512]-column chunks through a dedicated 1-bank
    psum slot (plus the psv ring in phase A), emitted into per-tt PE gaps
    ahead of their deadlines; DMA arrival order is tuned so the first exp
    fires at ~13 us.
  - normalize: DVE reciprocal of the ones-rows + tensor_mul into oT, split
    per sh so psum slots free progressively.
  - O-projection per (m, sh): 4 matmuls into a pv-ring psum slot + fused
    bias on the copy-out; frame-0 groups spread into frame-1 units, frame-1
    groups pre-staged in unit 15 / drained through the freed scores ring
    with the tail copies on the (by then idle) ScalarE.
  - y^T [C, S] per frame DMAed out; host un-transposes.
"""

import os
import sys

import numpy as np

try:
    import concourse  # noqa: F401
except ImportError:  # pragma: no cover
    for _p in ("/opt/trn_rl_repo", os.path.dirname(os.path.abspath(__file__))):
        if os.path.isdir(os.path.join(_p, "concourse")):
            sys.path.insert(0, _p)
            break

import concourse.mybir as mybir  # noqa: E402
import concourse.tile as tile  # noqa: E402
from concourse import bacc  # noqa: E402
from concourse import bass_utils  # noqa: E402

F32 = mybir.dt.float32
F32R = mybir.dt.float32r
BF16 = mybir.dt.bfloat16
F8 = mybir.dt.float8e4
DR = mybir.MatmulPerfMode.DoubleRow

S = 1024          # sequence length per frame
C = 320           # channels
H = 8             # heads
D = 40            # head dim
DP = 64           # padded head dim
CP = H * DP       # 512, padded channels
NCORES = 8
FPC = 2           # frames per core
SCALE = float(D) ** -0.5

CI = [(0, 128), (128, 128), (256, 64)]    # c_in chunks of 320
CO = [(0, 128), (128, 128), (256, 64)]    # c_out chunks of 320

# exp(s*SCALE) ~= (1 + y(c1 + y(c2 + y*c3)))^2 with y = s*SCALE/2 folded into
# the coefficients; runs on the (otherwise underused) DVE as a custom op so
# part of the softmax exp stream comes off the Activation-engine bottleneck.
_EXPC = (1.0024652, 0.51482491, 0.16152836)
_ALPHA = SCALE / 2.0
EXP_S0 = _EXPC[0] * _ALPHA
EXP_S1 = _EXPC[1] * _ALPHA * _ALPHA
EXP_IMM2 = _EXPC[2] * _ALPHA * _ALPHA * _ALPHA

# (u, tt) score tiles whose exp runs on DVE instead of Act
DVE_TILES = frozenset(
    (u, tt) for u in range(2, 14) for tt in (3, 7))

_OPS_CACHE: list = []


def _register_exp_op():
    if _OPS_CACHE:
        return _OPS_CACHE[0]
    import concourse.dve_ops as dve_ops
    from concourse.dve_spec import Spec, Src0, C0, C1, C2, One, sq

    for op in dve_ops.OPS:
        if op.name == "EXP_POLY_SQ_ANT":
            _OPS_CACHE.append(op)
            return op

    def _exp_ref(in0, in1, c0, c1, c2):
        x = in0.astype(np.float32)
        p = ((x * c2 + c1) * x + c0) * x + 1.0
        return (p * p).astype(np.float32)

    spec = Spec(
        body=sq(((Src0 * C2 + C1) * Src0 + C0) * Src0 + One),
        reference=_exp_ref,
    )
    dve_ops._SUB_OPCODE_FOR_NAME["EXP_POLY_SQ_ANT"] = (
        dve_ops._CUSTOM_DVE_ROW_BASE + len(dve_ops.OPS))
    op = dve_ops.DveOp("EXP_POLY_SQ_ANT", spec, False,
                       {"v3": "0d91af070d61a8d0"})
    dve_ops.OPS.append(op)
    dve_ops.CUSTOM_DVE_SPECS["EXP_POLY_SQ_ANT"] = spec
    _OPS_CACHE.append(op)
    return op


_NC_CACHE: dict = {}
LAST_RESULTS = None


def _build(loop_n: int = 1):
    exp_op = _register_exp_op()
    nc = bacc.Bacc("TRN2", target_bir_lowering=False, debug=False)

    CPAD = 384
    xt0 = nc.dram_tensor("xt0", [CPAD, S], BF16, kind="ExternalInput")
    xtf = nc.dram_tensor("xtf", [FPC, CPAD, S], BF16, kind="ExternalInput")
    wkq = nc.dram_tensor("wkq", [CPAD, CP], BF16, kind="ExternalInput")
    wqb = nc.dram_tensor("wqb", [CPAD, CP], BF16, kind="ExternalInput")
    wvp = nc.dram_tensor("wvp", [CPAD, CP], BF16, kind="ExternalInput")
    wo = nc.dram_tensor("wo", [CP, C], F32R, kind="ExternalInput")
    bo = nc.dram_tensor("bo", [128, 3], F32, kind="ExternalInput")
    yt = nc.dram_tensor("yt", [FPC, C, S], F32, kind="ExternalOutput")
    # fp8 q/k staging scratch: DMA round-trip repacks [2h x 64d, s] psum-row
    # order into the DoubleRow [32, kk, par, s] layout (slots: 4 k + 8 q)
    qk8d = nc.dram_tensor("qk8d", [12, 128, S], F8, kind="Internal")

    with tile.TileContext(nc) as tc:
        with (
            tc.tile_pool(name="pconst", bufs=1) as pconst,
            tc.tile_pool(name="pqk", bufs=1) as pqk,
            tc.tile_pool(name="pvs", bufs=1) as pvs,
            tc.tile_pool(name="pout", bufs=1) as pout,
            tc.tile_pool(name="pp8", bufs=4) as pp8,
            tc.tile_pool(name="prc", bufs=3) as prc,
            tc.tile_pool(name="py", bufs=6) as py,
            tc.tile_pool(name="psb", bufs=2, space="PSUM") as psb,   # scores ring
            tc.tile_pool(name="psv", bufs=3, space="PSUM") as psv,   # pv/y ring
            tc.tile_pool(name="psj", bufs=1, space="PSUM") as psj,   # proj slot
        ):
          for it in range(loop_n):
            P = f"{it}_"

            # ---------- persistent sbuf tiles ----------
            wkq_sb = pconst.tile([128, 3 * CP], BF16, name=f"{P}wkq", tag="wkq")
            wkq_v = wkq_sb[:].rearrange("p (c w) -> p c w", w=CP)
            wk_sb = [wkq_v[0:cn, ci] for ci, (cs, cn) in enumerate(CI)]
            # q-projection weights in bf16 to pair with the bf16 activations
            # (the compiler requires width-matched matmul inputs)
            wqb_sb = pconst.tile([128, 3 * CP], BF16, name=f"{P}wqb", tag="wqb")
            wqb_v = wqb_sb[:].rearrange("p (c w) -> p c w", w=CP)
            wq_sb = [wqb_v[0:cn, ci] for ci, (cs, cn) in enumerate(CI)]
            wv_all = pconst.tile([128, 3 * CP], BF16, name=f"{P}wv", tag="wv")
            wv_v = wv_all[:].rearrange("p (c w) -> p c w", w=CP)
            wv_sb = [wv_v[0:cn, ci] for ci, (cs, cn) in enumerate(CI)]
            x0_all = pconst.tile([128, 3 * S], BF16, name=f"{P}x0", tag="x0")
            x0_v = x0_all[:].rearrange("p (c w) -> p c w", w=S)
            x0_sb = [x0_v[0:cn, ci] for ci, (cs, cn) in enumerate(CI)]
            # q-side activations in bf16: q/k get fp8-quantized for the
            # DoubleRow scores anyway, so bf16 transport noise (~0.4%) is
            # negligible next to the 3% fp8 step; halves the startup DMAs
            xf_all = [
                pconst.tile([128, 3 * S], BF16, name=f"{P}xf{f}", tag=f"xf{f}")
                for f in range(FPC)
            ]
            xf_v = [xf_all[f][:].rearrange("p (c w) -> p c w", w=S) for f in range(FPC)]
            xf_sb = [
                [xf_v[f][0:cn, ci] for ci, (cs, cn) in enumerate(CI)]
                for f in range(FPC)
            ]
            wo_all = pconst.tile([128, 4 * C], F32R, name=f"{P}wo", tag="wo")
            wo_sb = [wo_all[:, cp * C:(cp + 1) * C] for cp in range(4)]
            bo_all = pconst.tile([128, 3], F32, name=f"{P}bo", tag="bo")
            bo_col = [bo_all[0:cn, m:m + 1] for m, (cs, cn) in enumerate(CO)]

            # fp8 packed q/k for DoubleRow scores: [32, kk2, par2, s1024]
            k8 = [pqk.tile([32, 4 * S], F8, name=f"{P}k8_{m}", tag=f"k8{m}") for m in range(4)]
            q8 = [
                [pqk.tile([32, 4 * S], F8, name=f"{P}q8_{f}_{m}", tag="q8", bufs=4) for m in range(4)]
                for f in range(FPC)
            ]
            k8v = [t[:].rearrange("p (kk par s) -> p kk par s", kk=2, par=2) for t in k8]
            q8v = [
                [t[:].rearrange("p (kk par s) -> p kk par s", kk=2, par=2) for t in q8[f]]
                for f in range(FPC)
            ]
            # f32r q/k for units 0-1 (head-pair 0 of frame 0): keeps the fp8
            # repack DMAs off the startup critical path
            kTp0 = pqk.tile([128, S], F32R, name=f"{P}kTp0", tag="kTp0")
            qTp00 = pqk.tile([128, S], F32R, name=f"{P}qTp00", tag="qTp00")
            # v (f32r, exact): [t128, h8 x (64v | 64 ones)] per t-chunk
            v_sb = [pvs.tile([128, H * 128], F32R, name=f"{P}v{tt}", tag=f"v{tt}")
                    for tt in range(8)]
            # PE warm-up tile memset must run FIRST on Pool so the dummy
            # matmuls can start ramping the p-state model immediately
            # (fp8: f32r memsets are invalid ISA; fp8 matmuls ramp the same)
            zwarm = pconst.tile([128, 512], F8, name=f"{P}zwarm", tag="zw")
            nc.gpsimd.memset(zwarm[:], 0.0)
            ones_sb = pconst.tile([128, DP], F32, name=f"{P}ones", tag="ones")
            nc.gpsimd.memset(ones_sb[:], 1.0)
            # ones blocks of v_sb filled once (DVE is idle in phase A)
            for tt in range(8):
                nc.vector.tensor_copy(
                    v_sb[tt][:].rearrange("p (h m) -> p h m", h=H)[:, :, DP:128],
                    ones_sb[:].unsqueeze(1).broadcast_to([128, H, DP]),
                )

            oT = [pout.tile([128, S], F32R, name=f"{P}oT{f}_{hp}", tag="oT", bufs=8)
                  for f in range(FPC) for hp in range(4)]
            oTf = [oT[0:4], oT[4:8]]

            # ---------- DMA emission (priority order) ----------
            def dma_cols(dst_view, dram_ap, c0, c1, width):
                """columns [c0:c1) of a [CPAD, width] dram tensor into the
                3-chunk sbuf view [128, 3, width]."""
                nc.sync.dma_start(
                    dst_view[:, :, c0:c1],
                    dram_ap.rearrange("(c p) w -> p c w", p=128)[:, :, c0:c1],
                )

            nc.sync.dma_start(bo_all[:], bo.ap())
            dma_cols(wqb_v, wqb.ap(), 0, 128, CP)              # wq head-pair 0
            dma_cols(wkq_v, wkq.ap(), 0, 128, CP)              # wk head-pair 0
            dma_cols(x0_v, xt0.ap(), 0, 128, S)                # x0 t 0:128 (tiny)
            dma_cols(xf_v[0], xtf.ap()[0], 0, 512, S)
            dma_cols(xf_v[0], xtf.ap()[0], 512, 1024, S)
            nc.sync.dma_start(wv_v, wvp.ap().rearrange("(c p) w -> p c w", p=128))
            dma_cols(x0_v, xt0.ap(), 128, 512, S)
            dma_cols(x0_v, xt0.ap(), 512, 1024, S)
            dma_cols(wkq_v, wkq.ap(), 128, CP, CP)             # wk rest
            dma_cols(wqb_v, wqb.ap(), 128, CP, CP)             # wq rest
            # xf1 and wo are deferred into unit 1-3 extras so the small fp8
            # repack hops for k8[1]/q8[0][1] aren't stuck behind their
            # transfers in the FIFO DMA queue

            # ---------- building blocks ----------
            def proj_chunk(dst8, dslot, w_tiles, x_tiles, m, sh, pool=None,
                           f32_dst=None, skip_pack=False):
                """One 512-col chunk of a q/k projection: matmul to psum, DVE
                copy to fp8 stage, then a DRAM round-trip that repacks psum-row
                order (par, kk, p) into the DoubleRow layout [32, kk, par, s].
                dst8: packed [32, kk, par, S] view; dslot: scratch dram slot.
                f32_dst: also (or only, with skip_pack) copy to an f32r tile."""
                ps = (pool or psj).tile(
                    [128, 512], F32, name=f"{P}pj{m}{sh}{dslot}",
                    tag="pj" if pool is None else "pv",
                )
                for ci in range(3):
                    nc.tensor.matmul(
                        ps[:],
                        w_tiles[ci][:, m * 128:(m + 1) * 128],
                        x_tiles[ci][:, sh * 512:(sh + 1) * 512],
                        start=(ci == 0),
                        stop=(ci == 2),
                    )
                cols = slice(sh * 512, (sh + 1) * 512)
                if f32_dst is not None:
                    nc.vector.tensor_copy(f32_dst[:, cols], ps[:])
                if skip_pack:
                    return
                stg = pqk.tile([128, 512], F8, name=f"{P}stg{m}{sh}{dslot}",
                               tag="stg", bufs=2)
                nc.vector.tensor_copy(stg[:], ps[:])
                nc.sync.dma_start(qk8d.ap()[dslot, :, cols], stg[:])
                for par in range(2):
                    nc.sync.dma_start(
                        dst8[:, :, par, cols],
                        qk8d.ap()[dslot, par * 64:(par + 1) * 64].rearrange(
                            "(kk p) s -> p kk s", kk=2)[:, :, cols],
                    )

            def vproj(tt, pool=None):
                """v_sb (f32r, exact) for t-chunk tt."""
                ps = (pool or psj).tile([128, 512], F32, name=f"{P}pv{tt}",
                                        tag="pj" if pool is None else "pv")
                for ci in range(3):
                    nc.tensor.matmul(
                        ps[:],
                        x0_sb[ci][:, tt * 128:(tt + 1) * 128],
                        wv_sb[ci][:],
                        start=(ci == 0),
                        stop=(ci == 2),
                    )
                nc.vector.tensor_copy(
                    v_sb[tt][:].rearrange("p (h m) -> p h m", h=H)[:, :, 0:DP],
                    ps[:].rearrange("p (h c) -> p h c", c=DP),
                )

            def oproj_start(f, m, sh):
                """first half of an O-proj group: psum + cp 0-1 matmuls."""
                cos, con = CO[m]
                ps = psv.tile([con, 512], F32, name=f"{P}py{f}{m}{sh}", tag="pv")
                for cp in range(2):
                    nc.tensor.matmul(
                        ps[:],
                        wo_sb[cp][:, cos:cos + con],
                        oTf[f][cp][:, sh * 512:(sh + 1) * 512],
                        start=(cp == 0),
                        stop=False,
                    )
                return ps

            def oproj_finish(f, m, sh, ps):
                cos, con = CO[m]
                for cp in range(2, 4):
                    nc.tensor.matmul(
                        ps[:],
                        wo_sb[cp][:, cos:cos + con],
                        oTf[f][cp][:, sh * 512:(sh + 1) * 512],
                        start=False,
                        stop=(cp == 3),
                    )
                y_sb = py.tile([con, 512], F32, name=f"{P}y{f}{m}{sh}", tag="y")
                nc.vector.tensor_scalar_add(y_sb[:], ps[:], bo_col[m])
                nc.sync.dma_start(
                    yt.ap()[f, cos:cos + con, sh * 512:(sh + 1) * 512], y_sb[:]
                )

            def oproj_group(f, m, sh):
                oproj_finish(f, m, sh, oproj_start(f, m, sh))

            # ---------- extras schedule: (unit, tt) -> list of closures ----------
            sched: dict = {}

            def at(u, tt, fn):
                sched.setdefault((u, tt), []).append(fn)

            # unit 0: v-projections (psv ring, pipelined 3-deep) and the
            # second half of kT head-pair 0
            at(0, 0, lambda: proj_chunk(k8v[0], 0, wk_sb, x0_sb, 0, 0,
                                        f32_dst=kTp0))
            at(0, 1, lambda: proj_chunk(k8v[0], 0, wk_sb, x0_sb, 0, 1,
                                        f32_dst=kTp0))
            at(0, 1, lambda: vproj(0, psv))
            at(0, 1, lambda: vproj(1, psv))
            at(0, 2, lambda: vproj(2, psv))
            at(0, 2, lambda: vproj(3, psv))
            at(0, 3, lambda: vproj(4, psv))
            at(0, 3, lambda: vproj(5, psv))
            at(0, 4, lambda: vproj(6, psv))
            at(0, 4, lambda: vproj(7, psv))
            at(1, 5, lambda: dma_cols(xf_v[1], xtf.ap()[1], 0, 512, S))
            at(2, 5, lambda: dma_cols(xf_v[1], xtf.ap()[1], 512, 1024, S))
            at(3, 5, lambda: nc.sync.dma_start(
                wo_all[:].rearrange("p (cp c) -> p cp c", c=C),
                wo.ap().rearrange("(cp p) c -> p cp c", p=128),
            ))
            # k/q projections for later units; deadline: unit 2*m (k8[m], q8[0][m]),
            # unit 8+2*m (q8[1][m])
            for m in range(1, 4):
                u0 = 2 * m - 2
                at(u0, 4, lambda m=m: proj_chunk(k8v[m], m, wk_sb, x0_sb, m, 0))
                at(u0, 6, lambda m=m: proj_chunk(k8v[m], m, wk_sb, x0_sb, m, 1))
                at(u0, 7, lambda m=m: proj_chunk(q8v[0][m], 4 + m, wq_sb, xf_sb[0], m, 0))
                at(u0 + 1, 1, lambda m=m: proj_chunk(q8v[0][m], 4 + m, wq_sb, xf_sb[0], m, 1))
            for m in range(4):
                u0 = 5 + 2 * m
                at(u0, 1, lambda m=m: proj_chunk(q8v[1][m], 8 + m, wq_sb, xf_sb[1], m, 0))
                at(u0, 3, lambda m=m: proj_chunk(q8v[1][m], 8 + m, wq_sb, xf_sb[1], m, 1))
            # frame-0 O-projection groups spread into frame-1 units (two halves
            # per group so no single PE insertion exceeds the per-tt slack)
            _ostate: dict = {}
            for g in range(6):
                m, sh = divmod(g, 2)
                at(8 + g, 3, lambda m=m, sh=sh: _ostate.__setitem__(
                    (m, sh), oproj_start(0, m, sh)))
                at(8 + g, 5, lambda m=m, sh=sh: oproj_finish(
                    0, m, sh, _ostate.pop((m, sh))))
            # frame-1 sh0 tail groups: pre-stage cp0-2 in unit 15's last slots
            _ytail: dict = {}

            def ypre(m, pool, tag):
                cos, con = CO[m]
                ps = pool.tile([con, 512], F32, name=f"{P}ypre{m}", tag=tag)
                for cp in range(3):
                    nc.tensor.matmul(
                        ps[:], wo_sb[cp][:, cos:cos + con], oTf[1][cp][:, 0:512],
                        start=(cp == 0), stop=False,
                    )
                _ytail[(m, 0)] = ps

            at(15, 5, lambda: ypre(0, psj, "pj"))
            at(15, 6, lambda: ypre(1, psb, "st"))

            # ---------- phase A: PE warm-up + minimal startup projections ----------
            # ~14 dummy matmuls on zeroed sbuf ramp the PE p-state model to
            # full clock while the input DMAs are still in flight; the real
            # phase-A chunks then cost 213 ns instead of 788 ns.
            pswarm = psj.tile([128, 512], F32, name=f"{P}pswarm", tag="pj")
            for _w in range(7):
                nc.tensor.matmul(pswarm[:], zwarm[:, 0:128], zwarm[:],
                                 start=True, stop=True, skip_group_check=True)
            # critical chain: kT head-pair-0 (t 0:512), q head-pair-0 (full s)
            # via f32r (fp8 pack for k8[0] rides along: only frame 1 needs it)
            proj_chunk(q8v[0][0], 4, wq_sb, xf_sb[0], 0, 0, pool=psv,
                       f32_dst=qTp00, skip_pack=True)
            ps_k0 = psj.tile([128, 128], F32, name=f"{P}pjk0t0", tag="pj")
            for ci in range(3):
                nc.tensor.matmul(ps_k0[:], wk_sb[ci][:, 0:128],
                                 x0_sb[ci][:, 0:128],
                                 start=(ci == 0), stop=(ci == 2))
            nc.vector.tensor_copy(kTp0[:, 0:128], ps_k0[:])
            proj_chunk(q8v[0][0], 4, wq_sb, xf_sb[0], 0, 1,
                       f32_dst=qTp00, skip_pack=True)

            # ---------- main unit loop ----------
            # PV emission for unit u is deferred by defer[u] tt-slots past the
            # pair's last exp: unit 0's v8/r8 conversions trail the first
            # scores, so its PVs (and transitively units 1-3, via the psum-
            # ring rotation) slip; from unit 4 on the schedule is the steady
            # state (PV right after each odd exp, normalize at unit end).
            defer = [5, 4, 3, 2] + [2] * 11 + [1]
            pv_plan: dict = {}
            for uu in range(16):
                for pp in range(4):
                    ue, te = divmod(8 * uu + 2 * pp + defer[uu], 8)
                    pv_plan.setdefault((ue, te), []).append((uu, pp))
            pvt: dict = {}
            p8t: dict = {}
            meta = [(*divmod(uu, H),) for uu in range(16)]  # (f, h)

            def norm_unit(usrc):
                fs, hs = meta[usrc]
                hps, pars = divmod(hs, 2)
                hls = pars * 64
                # one shared reciprocal tile: the sh1 recip's write-after-read
                # hazard on it forces the sh0 multiply to schedule first, which
                # shortens the critical normalize->O-projection chain
                rc = prc.tile([64, 512], F32, name=f"{P}rc{usrc}", tag="rc")
                for sh in range(2):
                    nc.vector.reciprocal(rc[:], pvt[usrc][sh][64:128, :])
                    nc.vector.tensor_mul(
                        oT[fs * 4 + hps][hls:hls + 64, sh * 512:(sh + 1) * 512],
                        pvt[usrc][sh][0:64, :],
                        rc[:],
                    )

            def emit_pv(usrc, pp):
                fs, hs = meta[usrc]
                if pp == 0:
                    pvt[usrc] = [
                        psv.tile([128, 512], F32, name=f"{P}pvac{usrc}{sh}", tag="pv")
                        for sh in range(2)
                    ]
                for kk in range(2):
                    tt = 2 * pp + kk
                    pt = p8t[(usrc, tt)]
                    for sh in range(2):
                        nc.tensor.matmul(
                            pvt[usrc][sh][:],
                            v_sb[tt][:, hs * 128:(hs + 1) * 128],
                            pt[:, sh * 512:(sh + 1) * 512],
                            start=(tt == 0),
                            stop=(tt == 7),
                        )
                if pp == 3:
                    norm_unit(usrc)

            def emit_scores(u, tt):
                f, h = divmod(u, H)
                hp, par = divmod(h, 2)
                hl = par * 64
                st = psb.tile([128, S], F32, name=f"{P}st{u}{tt}", tag="st")
                for sh in range(2):
                    if u < 2:
                        nc.tensor.matmul(
                            st[:, sh * 512:(sh + 1) * 512],
                            kTp0[hl:hl + 64, tt * 128:(tt + 1) * 128],
                            qTp00[hl:hl + 64, sh * 512:(sh + 1) * 512],
                            start=True,
                            stop=True,
                            tile_position=(hl, 0),
                        )
                    else:
                        nc.tensor.matmul(
                            st[:, sh * 512:(sh + 1) * 512],
                            k8v[hp][:, :, par, tt * 128:(tt + 1) * 128],
                            q8v[f][hp][:, :, par, sh * 512:(sh + 1) * 512],
                            start=True,
                            stop=True,
                            perf_mode=DR,
                            skip_group_check=True,
                        )
                return st

            # scores for slot i+1 are emitted right after exp(i) is issued
            # (before the PV/extras PE work), so the exp stream never waits
            # on late matmuls; the 2-deep psb ring supports exactly this.
            slots = [(u, tt) for u in range(16) for tt in range(8)]
            sc_next = emit_scores(*slots[0])
            for i, (u, tt) in enumerate(slots):
                st = sc_next
                pt = pp8.tile([128, S], F32R, name=f"{P}pt_{u}_{tt}",
                              tag="p8", bufs=6)
                p8t[(u, tt)] = pt
                if (u, tt) in DVE_TILES:
                    nc.vector._custom_dve(
                        exp_op, out=pt[:], in0=st[:],
                        s0=EXP_S0, s1=EXP_S1, imm2=EXP_IMM2,
                    )
                else:
                    nc.scalar.activation(
                        pt[:], st[:], mybir.ActivationFunctionType.Exp,
                        scale=SCALE,
                    )
                if (u, tt) in DVE_TILES and i + 1 < len(slots):
                    # run-ahead only across DVE-exp slots: the Act stream's
                    # next scores are ready the moment its previous exp ends
                    sc_next = emit_scores(*slots[i + 1])
                    run_ahead = True
                else:
                    run_ahead = False
                for fn in sched.pop((u, tt), ()):
                    fn()
                for usrc, ppe in pv_plan.pop((u, tt), ()):
                    emit_pv(usrc, ppe)
                if not run_ahead and i + 1 < len(slots):
                    sc_next = emit_scores(*slots[i + 1])

            # ---------- frame-1 O-projection tail ----------
            # sh0 groups were pre-staged (cp0-2) during unit 15; add the cp3
            # matmul (gated on the last normalize) and drain. sh1 groups run
            # full-size through the freed scores ring.
            # sh1 groups: cp0-2 pre-staged immediately through the freeing psv
            # ring (hp0-2 oT halves have been ready since unit 13)
            ytail1 = []
            for m in range(3):
                cos, con = CO[m]
                ps = psv.tile([con, 512], F32, name=f"{P}pyt1{m}1", tag="pv")
                for cp in range(3):
                    nc.tensor.matmul(
                        ps[:], wo_sb[cp][:, cos:cos + con],
                        oTf[1][cp][:, 512:1024],
                        start=(cp == 0), stop=False,
                    )
                ytail1.append(ps)
            # sh0 drains
            for m in range(2):
                cos, con = CO[m]
                ps = _ytail.pop((m, 0))
                nc.tensor.matmul(
                    ps[:], wo_sb[3][:, cos:cos + con], oTf[1][3][:, 0:512],
                    start=False, stop=True,
                )
                y_sb = py.tile([con, 512], F32, name=f"{P}yt1{m}0", tag="y")
                if m == 0:
                    nc.scalar.activation(y_sb[:], ps[:],
                                         mybir.ActivationFunctionType.Identity,
                                         bias=bo_col[m])
                else:
                    nc.vector.tensor_scalar_add(y_sb[:], ps[:], bo_col[m])
                nc.sync.dma_start(yt.ap()[1, cos:cos + con, 0:512], y_sb[:])
            cos, con = CO[2]
            ps = psb.tile([con, 512], F32, name=f"{P}pyt120", tag="st")
            for cp in range(4):
                nc.tensor.matmul(
                    ps[:], wo_sb[cp][:, cos:cos + con], oTf[1][cp][:, 0:512],
                    start=(cp == 0), stop=(cp == 3),
                )
            y_sb = py.tile([con, 512], F32, name=f"{P}yt120", tag="y")
            nc.scalar.activation(y_sb[:], ps[:],
                                 mybir.ActivationFunctionType.Identity,
                                 bias=bo_col[2])
            nc.sync.dma_start(yt.ap()[1, cos:cos + con, 0:512], y_sb[:])
            # sh1 drains
            for m in range(3):
                cos, con = CO[m]
                ps = ytail1[m]
                nc.tensor.matmul(
                    ps[:], wo_sb[3][:, cos:cos + con], oTf[1][3][:, 512:1024],
                    start=False, stop=True,
                )
                y_sb = py.tile([con, 512], F32, name=f"{P}yt1{m}1", tag="y")
                if m % 2 == 0:
                    nc.scalar.activation(y_sb[:], ps[:],
                                         mybir.ActivationFunctionType.Identity,
                                         bias=bo_col[m])
                else:
                    nc.vector.tensor_scalar_add(y_sb[:], ps[:], bo_col[m])
                nc.sync.dma_start(yt.ap()[1, cos:cos + con, 512:1024], y_sb[:])

            assert not sched, f"unscheduled extras: {list(sched)}"
            assert not _ytail, f"unfinished tail groups: {list(_ytail)}"

    nc.compile()
    return nc


def _get_nc(loop_n: int = 1):
    if loop_n not in _NC_CACHE:
        _NC_CACHE[loop_n] = _build(loop_n)
    return _NC_CACHE[loop_n]


_BF16_NP = mybir.dt.np(mybir.dt.bfloat16)


def _pad_heads_cols(wT: np.ndarray) -> np.ndarray:
    """[C, C] (c_in, c_out) -> [C, CP] with each head's 40 cols at h*64."""
    out = np.zeros((C, CP), np.float32)
    out.reshape(C, H, DP)[:, :, :D] = wT.reshape(C, H, D)
    return out


def _prep_inputs(hidden_states, Wq, Wk, Wv, Wo, bo, video_length, k):
    hidden_states = np.asarray(hidden_states, dtype=np.float32)
    B = hidden_states.shape[0]
    assert hidden_states.shape == (B, S, C), hidden_states.shape
    assert B == NCORES * FPC, B
    kf = int(k)
    vl = int(video_length)
    b = B // vl
    assert b == 1, "kernel specialized for batch 1 (b*video_length == B)"

    xT = np.zeros((B, 384, S), np.float32)
    xT[:, :C, :] = hidden_states.transpose(0, 2, 1)
    wk_p = _pad_heads_cols(np.asarray(Wk, np.float32).T)
    wq_p = _pad_heads_cols(np.asarray(Wq, np.float32).T)
    wv_p = _pad_heads_cols(np.asarray(Wv, np.float32).T)
    wkq_p = np.zeros((384, CP), _BF16_NP)
    wkq_p[:C] = wk_p.astype(_BF16_NP)
    wqb_p = np.zeros((384, CP), _BF16_NP)
    wqb_p[:C] = wq_p.astype(_BF16_NP)
    wvp_p = np.zeros((384, CP), _BF16_NP)
    wvp_p[:C] = wv_p.astype(_BF16_NP)
    wo_p = np.zeros((CP, C), np.float32)
    wo_p.reshape(H, DP, C)[:, :D, :] = np.asarray(Wo, np.float32).T.reshape(H, D, C)
    bo_f = np.zeros(384, np.float32)
    bo_f[:C] = np.asarray(bo, np.float32)
    bo_t = np.ascontiguousarray(bo_f.reshape(3, 128).T)

    xt0 = np.ascontiguousarray(xT[kf].astype(_BF16_NP))
    in_maps = []
    for c in range(NCORES):
        in_maps.append(
            {
                "xt0": xt0,
                "xtf": np.ascontiguousarray(
                    xT[c * FPC:(c + 1) * FPC].astype(_BF16_NP)),
                "wkq": wkq_p,
                "wqb": wqb_p,
                "wvp": wvp_p,
                "wo": wo_p,
                "bo": bo_t,
            }
        )
    return in_maps


def _run(inputs: dict, loop_n: int = 1):
    global LAST_RESULTS
    nc = _get_nc(loop_n)
    in_maps = _prep_inputs(**inputs)
    last_exc = None
    for _attempt in range(3):
        try:
            res = bass_utils.run_bass_kernel_spmd(nc, in_maps, core_ids=list(range(NCORES)))
            break
        except Exception as e:  # transient NRT/axon device hiccups
            last_exc = e
            import time as _time
            _time.sleep(2.0)
    else:
        raise last_exc
    LAST_RESULTS = res
    B = NCORES * FPC
    y = np.empty((B, S, C), np.float32)
    for c in range(NCORES):
        y[c * FPC:(c + 1) * FPC] = res.results[c]["yt"].transpose(0, 2, 1)
    return y


def kernel(hidden_states, Wq, Wk, Wv, Wo, bo, video_length, k):
    return _run(
        dict(
            hidden_states=hidden_states,
            Wq=Wq,
            Wk=Wk,
            Wv=Wv,
            Wo=Wo,
            bo=bo,
            video_length=video_length,
            k=k,
        )
    )

